# revision 3
# baseline (speedup 1.0000x reference)
"""Causal self-attention (dense transformer) on 8 TRN2 NeuronCores.

Sharding: heads+batch tensor-parallel. Each core c owns 2 heads (2c, 2c+1)
for all 4 batches:
  - QKV projection with the w_qkv row-slice for its heads (x is replicated,
    fed pre-transposed as xT [d, tokens] so d lands on SBUF partitions).
  - Causal attention for its 8 (batch, head) pairs, computed in
    "transposed scores" layout S_t[tk, tq]. PV matmuls are token-major
    (stationary P^T tile, moving [V | 1]) so the output free dim is 65 wide
    instead of 512: y accumulates per 128-token q-tile as [tq, hd+1] with the
    softmax denominator in the last column.
  - Normalization is a per-partition reciprocal (DVE) + broadcast multiply
    (Pool) into a token-major y_tok buffer.
  - Per-batch AllToAll exchanges y token-major; the receiver DMA-transposes
    each [tok, fl] chunk back to feature-major y_loc for the output
    projection (last batch exchanges in two halves so its collective
    overlaps).
Host side: shard/transpose/cast inputs, concat+transpose the output.

Compute dtype bf16 (PSUM accumulation fp32), storage fp32 in/out.
"""

import numpy as np
import ml_dtypes

import concourse.bass as bass
import concourse.mybir as mybir
import concourse.tile as tile
from concourse.bass_utils import run_bass_kernel_spmd

BF16 = mybir.dt.bfloat16
F32 = mybir.dt.float32
AF = mybir.ActivationFunctionType

# Full-size problem constants (hardcoded per harness contract)
N_CORES = 8
BSZ, SEQ, D, N_HEAD = 4, 2048, 1024, 16
HD = 64  # head dim


def _split_multi_waits(nc):
    """walrus on this build accepts at most ONE sync-wait command per
    instruction. Hoist extra waits onto standalone same-engine nops placed
    immediately before the instruction (queue order preserves semantics)."""
    edits = []
    for func in nc.m.functions:
        for bb in func.blocks:
            insts = bb.instructions
            for idx, ins in enumerate(insts):
                si = ins.sync_info
                if si is not None and len(si.on_wait) > 1:
                    edits.append((bb, idx, ins))
    for bb, idx, ins in reversed(edits):
        si = ins.sync_info
        extra, keep = list(si.on_wait[:-1]), [si.on_wait[-1]]
        ins.sync_info = mybir.SyncInfo(on_wait=keep, on_update=list(si.on_update))
        nops = []
        for w in extra:
            nop = nc.engines[ins.engine].nop().ins
            host = nc.cur_bb.bb.instructions
            assert host[-1] is nop
            host.pop()
            nop.sync_info = mybir.SyncInfo(on_wait=[w], on_update=[])
            nops.append(nop)
        live = bb.instructions
        for j, nop in enumerate(nops):
            live.insert(idx + j, nop)


def build_nc(n_cores=N_CORES, bsz=BSZ, seq=SEQ, d=D, n_head=N_HEAD):
    hd = HD
    hpc = n_head // n_cores          # heads per core
    fl = hpc * hd                    # local feature width (q/k/v per core)
    T = bsz * seq                    # total tokens
    kd = d // 128                    # contraction tiles over d
    tb = min(512, seq)               # tq block width (matmul free dim)
    nqb = seq // tb                  # q-blocks per batch
    dtiles = tb // 128               # 128-tiles per q-block
    nt = T // 128                    # total 128-token tiles
    scale = float(1.0 / np.sqrt(hd))

    tsb = seq // n_cores             # per-batch token chunk per core
    qpb = seq // 128                 # 128-token q-tiles per batch
    nc = bass.Bass(num_devices=n_cores)
    xT = nc.declare_dram_parameter("xT", [d, T], BF16, isOutput=False)
    wqkv = nc.declare_dram_parameter("wqkv", [d, 3 * fl], BF16, isOutput=False)
    wproj = nc.declare_dram_parameter("wproj", [d, d], BF16, isOutput=False)
    out = nc.declare_dram_parameter("out", [d, bsz * tsb], F32, isOutput=True)
    split_last = (nqb == 4)          # last batch exchanges in 2 halves
    tsh = tsb // 2
    a2a_in = [nc.dram_tensor(f"a2a_in{b}", [n_cores, 128, tsb], BF16)
              for b in range(bsz - 1 if split_last else bsz)]
    a2a_out = [nc.dram_tensor(f"a2a_out{b}", [n_cores, 128, tsb], BF16)
               for b in range(bsz - 1 if split_last else bsz)]
    if split_last:
        a2ah_in = [nc.dram_tensor(f"a2ah_in{h}", [n_cores, 128, tsh], BF16)
                   for h in range(2)]
        a2ah_out = [nc.dram_tensor(f"a2ah_out{h}", [n_cores, 128, tsh], BF16)
                    for h in range(2)]

    with tile.TileContext(nc) as tc:
        with (
            tc.tile_pool(name="const", bufs=1) as const,
            tc.tile_pool(name="xin", bufs=2) as xin,
            tc.tile_pool(name="work", bufs=3) as work,
            tc.tile_pool(name="psum", bufs=1, space="PSUM") as psum,
        ):
            # ---- persistent SBUF ----
            w_sb = [const.tile([128, 3 * fl], BF16, name=f"w_sb{i}") for i in range(kd)]
            for i in range(kd):
                nc.sync.dma_start(w_sb[i][:], wqkv[i * 128:(i + 1) * 128, :])
            wp_sb = [const.tile([fl, d], BF16, name=f"wp_sb{i}") for i in range(n_cores)]

            q_sb = const.tile([fl, T], BF16, name="q_sb")
            k_sb = const.tile([fl, T], BF16, name="k_sb")
            # token-major y: [tok128, (global qtile, fl)]
            y_tok = const.tile([128, nt * fl], BF16, name="y_tok")
            # v token-major with a ones column per (tile, head): tile g, head h
            # occupies cols g*hpc*(hd+1) + h*(hd+1) + [0, hd+1); col hd is ones.
            vw = hpc * (hd + 1)
            v_sb = const.tile([128, nt * vw], BF16, name="v_sb")
            ones_ap = v_sb.rearrange("p (n h c) -> p n h c", h=hpc,
                                     c=hd + 1)[:, :, :, hd:hd + 1]
            nc.vector.memset(ones_ap, 1.0)

            # ---- unified (batch, q-block) stream with qkv prefetch ----
            def qkv_block(b, qb):
                tbi = b * (seq // tb) + qb
                ts0 = tbi * tb
                x_t = xin.tile([128, kd * tb], BF16, name="x_t", tag="x", bufs=3)
                # one DMA: in [kd, 128, tb] (d-tiles) -> out [128, kd, tb]
                nc.sync.dma_start(
                    x_t[:].rearrange("p (i t) -> p i t", i=kd),
                    xT.rearrange("(i p) T -> p i T", p=128)[:, :, ts0:ts0 + tb])
                # q, k (feature-major): out [fl, tb]
                for which, dst in ((0, q_sb), (1, k_sb)):
                    ps = psum.tile([fl, tb], F32, name=f"ps_qk{which}", tag="mm512", bufs=2)
                    for i in range(kd):
                        nc.tensor.matmul(
                            ps[:], w_sb[i][:, which * fl:(which + 1) * fl],
                            x_t[:, i * tb:(i + 1) * tb],
                            start=(i == 0), stop=(i == kd - 1))
                    nc.vector.tensor_copy(dst[:, ts0:ts0 + tb], ps[:])
                # v (token-major): out [128 tok, fl]
                for tt in range(dtiles):
                    gti = tbi * dtiles + tt
                    ps_v = psum.tile([128, fl], F32, name="ps_v", tag="mm512", bufs=2)
                    for i in range(kd):
                        nc.tensor.matmul(
                            ps_v[:], x_t[:, i * tb + tt * 128:i * tb + (tt + 1) * 128],
                            w_sb[i][:, 2 * fl:3 * fl],
                            start=(i == 0), stop=(i == kd - 1))
                    nc.vector.tensor_copy(
                        v_sb.rearrange("p (n h c) -> p n h c", h=hpc,
                                       c=hd + 1)[:, gti, :, 0:hd],
                        ps_v[:].rearrange("p (h c) -> p h c", c=hd))

            def attn_block(b, qb):
                tq0 = b * seq + qb * tb
                ntk = (qb + 1) * dtiles
                # y accumulators: 2 one-bank tiles, 4 groups (2 qt x 2 heads)
                # each; group (qt, h) at cols (qt%2)*130 + h*65, width 65
                # (64 y + 1 denom). All matmuls accumulate (start=False) onto
                # an explicit memset so group order is irrelevant.
                ya = [psum.tile([128, 512], F32, name=f"ya{t}", tag=f"yacc{t}",
                                bufs=1) for t in range(2)]
                nc.vector.memset(ya[0][:, 0:2 * 130], 0.0)
                nc.vector.memset(ya[1][:, 0:2 * 130], 0.0)
                for tki in range(ntk):
                    t0 = b * seq + tki * 128
                    gti = t0 // 128
                    m = tki - qb * dtiles
                    # diagonal tile m covers only tq columns >= 128*m
                    c0 = 128 * m if m > 0 else 0
                    # both heads' scores side by side in one 2-bank tile
                    ps_s = psum.tile([128, hpc * tb], F32, name="ps_s",
                                     tag="s2", bufs=2)
                    p_t = work.tile([128, hpc * tb], BF16, name="p_t",
                                    tag="pt", bufs=6)
                    for h in range(hpc):
                        hs = slice(h * hd, (h + 1) * hd)
                        nc.tensor.matmul(ps_s[:, h * tb + c0:(h + 1) * tb],
                                         k_sb[hs, t0:t0 + 128],
                                         q_sb[hs, tq0 + c0:tq0 + tb],
                                         start=True, stop=True)
                    # one exp for both heads: AP [128, hpc, nq]
                    sv = ps_s[:].rearrange("p (h q) -> p h q", h=hpc)[:, :, c0:tb]
                    pv = p_t[:].rearrange("p (h q) -> p h q", h=hpc)[:, :, c0:tb]
                    nc.scalar.activation(pv, sv, AF.Exp, scale=scale)
                    if m >= 0:
                        # mask the [128,128] triangle at cols [c0, c0+128)
                        for h in range(hpc):
                            ap = p_t[:, h * tb + c0:h * tb + c0 + 128]
                            nc.gpsimd.affine_select(
                                out=ap, in_=ap,
                                compare_op=mybir.AluOpType.is_ge, fill=0.0,
                                base=0, channel_multiplier=-1,
                                pattern=[[1, 128]])
                    for h in range(hpc):
                        vs = v_sb[:, gti * vw + h * (hd + 1):
                                  gti * vw + (h + 1) * (hd + 1)]
                        for qt in range(max(0, m), dtiles):
                            ti, g = divmod(qt, 2)
                            off = g * 130 + h * 65
                            nc.tensor.matmul(
                                ya[ti][:, off:off + 65],
                                p_t[:, h * tb + qt * 128:h * tb + (qt + 1) * 128],
                                vs, start=False,
                                stop=(tki == qb * dtiles + qt),
                                skip_group_check=True)
                # normalize: y_tok[:, gq, h*hd:...] = y/denom (token-major)
                for qt in range(dtiles):
                    ti, g = divmod(qt, 2)
                    gq = b * qpb + qb * dtiles + qt
                    for h in range(hpc):
                        off = g * 130 + h * 65
                        rcp = work.tile([128, 1], F32, name="rcp", tag="rcp",
                                        bufs=8)
                        nc.vector.reciprocal(rcp[:], ya[ti][:, off + 64:off + 65])
                        nc.vector.tensor_scalar_mul(
                            y_tok[:, gq * fl + h * hd:gq * fl + (h + 1) * hd],
                            ya[ti][:, off:off + 64], rcp[:])

            def a2a_issue(b):
                # shard j of batch b = qtiles b*qpb + [2j, 2j+2)
                for j in range(n_cores):
                    g0 = b * qpb + 2 * j
                    nc.gpsimd.dma_start(
                        a2a_in[b][j],
                        y_tok[:, g0 * fl:(g0 + 2) * fl])
                nc.gpsimd.collective_compute(
                    "AllToAll", mybir.AluOpType.bypass,
                    replica_groups=[list(range(n_cores))],
                    ins=[a2a_in[b][:]], outs=[a2a_out[b][:]],
                )

            def a2a_issue_half(b, hf):
                # shard j of half hf = qtile b*qpb + hf*8 + j
                for j in range(n_cores):
                    g0 = b * qpb + hf * (qpb // 2) + j
                    nc.gpsimd.dma_start(
                        a2ah_in[hf][j],
                        y_tok[:, g0 * fl:(g0 + 1) * fl])
                nc.gpsimd.collective_compute(
                    "AllToAll", mybir.AluOpType.bypass,
                    replica_groups=[list(range(n_cores))],
                    ins=[a2ah_in[hf][:]], outs=[a2ah_out[hf][:]],
                )

            def proj_half(pb, hf):
                # output cols [pb*tsb + hf*tsh, pb*tsb + (hf+1)*tsh)
                y_loc = [work.tile([fl, tsh], BF16, name="y_loch",
                                   tag=f"ylh{i}", bufs=1) for i in range(n_cores)]
                for i in range(n_cores):
                    nc.sync.dma_start_transpose(y_loc[i][:], a2ah_out[hf][i])
                c0o = pb * tsb + hf * tsh
                for dj in range(d // 128):
                    ps_o = psum.tile([128, tsh], F32, name="ps_oh", tag="mm512", bufs=2)
                    for i in range(n_cores):
                        nc.tensor.matmul(
                            ps_o[:], wp_sb[i][:, dj * 128:(dj + 1) * 128],
                            y_loc[i][:], start=(i == 0), stop=(i == n_cores - 1))
                    o_sb = work.tile([128, tsh], F32, name="o_sbh", tag="osb", bufs=3)
                    nc.vector.tensor_copy(o_sb[:], ps_o[:])
                    nc.sync.dma_start(
                        out[dj * 128:(dj + 1) * 128, c0o:c0o + tsh], o_sb[:])

            def proj(pb):
                y_loc = [work.tile([fl, tsb], BF16, name="y_loc",
                                   tag=f"yloc{i}", bufs=2) for i in range(n_cores)]
                for i in range(n_cores):
                    for t in range(2):
                        nc.sync.dma_start_transpose(
                            y_loc[i][:, t * 128:(t + 1) * 128],
                            a2a_out[pb][i][:, t * 128:(t + 1) * 128])
                for dj in range(d // 128):
                    ps_o = psum.tile([128, tsb], F32, name="ps_o", tag="mm512", bufs=2)
                    for i in range(n_cores):
                        nc.tensor.matmul(
                            ps_o[:], wp_sb[i][:, dj * 128:(dj + 1) * 128],
                            y_loc[i][:], start=(i == 0), stop=(i == n_cores - 1))
                    o_sb = work.tile([128, tsb], F32, name="o_sb", tag="osb", bufs=3)
                    nc.vector.tensor_copy(o_sb[:], ps_o[:])
                    nc.sync.dma_start(
                        out[dj * 128:(dj + 1) * 128, pb * tsb:(pb + 1) * tsb],
                        o_sb[:])

            units = [(b, qb) for b in range(bsz) for qb in range(nqb)]
            qkv_block(*units[0])
            if len(units) > 1:
                qkv_block(*units[1])
            for i in range(n_cores):
                nc.sync.dma_start(wp_sb[i][:], wproj[i * fl:(i + 1) * fl, :])
            last_b = bsz - 1
            for L, (b, qb) in enumerate(units):
                attn_block(b, qb)
                if L + 2 < len(units):
                    qkv_block(*units[L + 2])
                if split_last and b == last_b and qb == nqb // 2 - 1:
                    a2a_issue_half(b, 0)
                if qb == nqb - 1:
                    if split_last and b == last_b:
                        a2a_issue_half(b, 1)
                    else:
                        a2a_issue(b)
                    if b >= 1:
                        proj(b - 1)
            if split_last:
                proj_half(last_b, 0)
                proj_half(last_b, 1)
            else:
                proj(bsz - 1)
    _split_multi_waits(nc)
    return nc


def shard_inputs(x, w_qkv, w_proj, n_cores=N_CORES, n_head=N_HEAD):
    bf16 = ml_dtypes.bfloat16
    d = x.shape[-1]
    T = x.shape[0] * x.shape[1]
    hpc = n_head // n_cores
    fl = hpc * HD
    xT = np.ascontiguousarray(np.asarray(x, np.float32).reshape(T, d).T.astype(bf16))
    wq = np.asarray(w_qkv, np.float32)
    wp = np.ascontiguousarray(np.asarray(w_proj, np.float32).T.astype(bf16))
    in_maps = []
    for c in range(n_cores):
        r0 = c * fl
        wqkv_c = np.ascontiguousarray(
            np.concatenate([wq[r0:r0 + fl], wq[d + r0:d + r0 + fl],
                            wq[2 * d + r0:2 * d + r0 + fl]], axis=0).T.astype(bf16))
        in_maps.append({"xT": xT, "wqkv": wqkv_c, "wproj": wp})
    return in_maps


def assemble_out(outs, n_cores=N_CORES, bsz=BSZ, seq=SEQ, d=D):
    """outs[c] is [d, bsz*tsb]; column block b holds tokens
    b*seq + [c*tsb, (c+1)*tsb) — except the last batch when nqb==4, which is
    exchanged in halves: tokens b*seq + h*seq/2 + [c*tsh, (c+1)*tsh)."""
    tsb = seq // n_cores
    tsh = tsb // 2
    split_last = (seq // min(512, seq)) == 4
    T = bsz * seq
    outT = np.empty((d, T), np.float32)
    for c in range(n_cores):
        for b in range(bsz):
            if split_last and b == bsz - 1:
                for h in range(2):
                    base = b * seq + h * (seq // 2)
                    outT[:, base + c * tsh:base + (c + 1) * tsh] = \
                        outs[c][:, b * tsb + h * tsh:b * tsb + (h + 1) * tsh]
            else:
                outT[:, b * seq + c * tsb:b * seq + (c + 1) * tsb] = \
                    outs[c][:, b * tsb:(b + 1) * tsb]
    return np.ascontiguousarray(outT.T).reshape(bsz, seq, d)


_NC_CACHE = {}


def kernel(x, w_qkv, w_proj):
    key = "full"
    if key not in _NC_CACHE:
        _NC_CACHE[key] = build_nc()
    nc = _NC_CACHE[key]
    in_maps = shard_inputs(x, w_qkv, w_proj)
    res = run_bass_kernel_spmd(nc, in_maps, list(range(N_CORES))).results
    return assemble_out([res[c]["out"] for c in range(N_CORES)]).astype(np.float32)


# revision 6
# speedup vs baseline: 1.1025x; 1.1025x over previous
"""Causal self-attention (dense transformer) on 8 TRN2 NeuronCores.

Sharding: heads+batch tensor-parallel. Each core c owns 2 heads (2c, 2c+1)
for all 4 batches:
  - QKV projection with the w_qkv row-slice for its heads (x is replicated,
    fed pre-transposed as xT [d, tokens] so d lands on SBUF partitions).
  - Causal attention for its 8 (batch, head) pairs, computed in
    "transposed scores" layout S_t[tk, tq]. PV matmuls are token-major
    (stationary P^T tile, moving [V | 1]) so the output free dim is 65 wide
    instead of 512: y accumulates per 128-token q-tile as [tq, hd+1] with the
    softmax denominator in the last column.
  - Normalization is a per-partition reciprocal (DVE) + broadcast multiply
    (Pool) into a token-major y_tok buffer.
  - Per-batch AllToAll exchanges y token-major; the receiver DMA-transposes
    each [tok, fl] chunk back to feature-major y_loc for the output
    projection (last batch exchanges in two halves so its collective
    overlaps).
Host side: shard/transpose/cast inputs, concat+transpose the output.

Compute dtype bf16 (PSUM accumulation fp32), storage fp32 in/out.
"""

import numpy as np
import ml_dtypes

import concourse.bass as bass
import concourse.mybir as mybir
import concourse.tile as tile
from concourse.bass_utils import run_bass_kernel_spmd

BF16 = mybir.dt.bfloat16
F32 = mybir.dt.float32
AF = mybir.ActivationFunctionType

# Full-size problem constants (hardcoded per harness contract)
N_CORES = 8
BSZ, SEQ, D, N_HEAD = 4, 2048, 1024, 16
HD = 64  # head dim


def _split_multi_waits(nc):
    """walrus on this build accepts at most ONE sync-wait command per
    instruction. Hoist extra waits onto standalone same-engine nops placed
    immediately before the instruction (queue order preserves semantics)."""
    edits = []
    for func in nc.m.functions:
        for bb in func.blocks:
            insts = bb.instructions
            for idx, ins in enumerate(insts):
                si = ins.sync_info
                if si is not None and len(si.on_wait) > 1:
                    edits.append((bb, idx, ins))
    for bb, idx, ins in reversed(edits):
        si = ins.sync_info
        extra, keep = list(si.on_wait[:-1]), [si.on_wait[-1]]
        ins.sync_info = mybir.SyncInfo(on_wait=keep, on_update=list(si.on_update))
        nops = []
        for w in extra:
            nop = nc.engines[ins.engine].nop().ins
            host = nc.cur_bb.bb.instructions
            assert host[-1] is nop
            host.pop()
            nop.sync_info = mybir.SyncInfo(on_wait=[w], on_update=[])
            nops.append(nop)
        live = bb.instructions
        for j, nop in enumerate(nops):
            live.insert(idx + j, nop)


def build_nc(n_cores=N_CORES, bsz=BSZ, seq=SEQ, d=D, n_head=N_HEAD):
    hd = HD
    hpc = n_head // n_cores          # heads per core
    fl = hpc * hd                    # local feature width (q/k/v per core)
    T = bsz * seq                    # total tokens
    kd = d // 128                    # contraction tiles over d
    tb = min(512, seq)               # tq block width (matmul free dim)
    nqb = seq // tb                  # q-blocks per batch
    dtiles = tb // 128               # 128-tiles per q-block
    nt = T // 128                    # total 128-token tiles
    scale = float(1.0 / np.sqrt(hd))

    tsb = seq // n_cores             # per-batch token chunk per core
    qpb = seq // 128                 # 128-token q-tiles per batch
    nc = bass.Bass(num_devices=n_cores)
    xT = nc.declare_dram_parameter("xT", [d, T], BF16, isOutput=False)
    wqkv = nc.declare_dram_parameter("wqkv", [d, 3 * fl], BF16, isOutput=False)
    wproj = nc.declare_dram_parameter("wproj", [d, d], BF16, isOutput=False)
    out = nc.declare_dram_parameter("out", [d, bsz * tsb], F32, isOutput=True)
    split_last = (nqb == 4)          # last batch exchanges in 2 halves
    tsh = tsb // 2
    a2a_in = [nc.dram_tensor(f"a2a_in{b}", [n_cores, 128, tsb], BF16)
              for b in range(bsz - 1 if split_last else bsz)]
    a2a_out = [nc.dram_tensor(f"a2a_out{b}", [n_cores, 128, tsb], BF16)
               for b in range(bsz - 1 if split_last else bsz)]
    if split_last:
        a2ah_in = [nc.dram_tensor(f"a2ah_in{h}", [n_cores, 128, tsh], BF16)
                   for h in range(2)]
        a2ah_out = [nc.dram_tensor(f"a2ah_out{h}", [n_cores, 128, tsh], BF16)
                    for h in range(2)]

    with tile.TileContext(nc) as tc:
        with (
            tc.tile_pool(name="const", bufs=1) as const,
            tc.tile_pool(name="xin", bufs=2) as xin,
            tc.tile_pool(name="work", bufs=3) as work,
            tc.tile_pool(name="psum", bufs=1, space="PSUM") as psum,
        ):
            # ---- persistent SBUF ----
            w_sb = [const.tile([128, 3 * fl], BF16, name=f"w_sb{i}") for i in range(kd)]
            for i in range(kd):
                nc.sync.dma_start(w_sb[i][:], wqkv[i * 128:(i + 1) * 128, :])
            wp_sb = [const.tile([fl, d], BF16, name=f"wp_sb{i}") for i in range(n_cores)]

            q_sb = const.tile([fl, T], BF16, name="q_sb")
            k_sb = const.tile([fl, T], BF16, name="k_sb")
            # token-major y: [tok128, (global qtile, fl)]
            y_tok = const.tile([128, nt * fl], BF16, name="y_tok")
            # v token-major with a ones column per (tile, head): tile g, head h
            # occupies cols g*hpc*(hd+1) + h*(hd+1) + [0, hd+1); col hd is ones.
            vw = hpc * (hd + 1)
            v_sb = const.tile([128, nt * vw], BF16, name="v_sb")
            ones_ap = v_sb.rearrange("p (n h c) -> p n h c", h=hpc,
                                     c=hd + 1)[:, :, :, hd:hd + 1]
            nc.vector.memset(ones_ap, 1.0)

            # triangular mask [128,128]: keep S_t[tk_i, tq_j] iff i <= j
            tri = const.tile([128, 128], BF16, name="tri")
            nc.gpsimd.memset(tri[:], 1.0)
            nc.gpsimd.affine_select(
                out=tri[:], in_=tri[:],
                compare_op=mybir.AluOpType.is_ge, fill=0.0,
                base=0, channel_multiplier=-1, pattern=[[1, 128]],
            )

            # ---- unified (batch, q-block) stream with qkv prefetch ----
            def qkv_block(b, qb):
                tbi = b * (seq // tb) + qb
                ts0 = tbi * tb
                x_t = xin.tile([128, kd * tb], BF16, name="x_t", tag="x", bufs=3)
                # one DMA: in [kd, 128, tb] (d-tiles) -> out [128, kd, tb]
                nc.sync.dma_start(
                    x_t[:].rearrange("p (i t) -> p i t", i=kd),
                    xT.rearrange("(i p) T -> p i T", p=128)[:, :, ts0:ts0 + tb])
                # q, k (feature-major): out [fl, tb]
                for which, dst in ((0, q_sb), (1, k_sb)):
                    ps = psum.tile([fl, tb], F32, name=f"ps_qk{which}", tag="mm512", bufs=2)
                    for i in range(kd):
                        nc.tensor.matmul(
                            ps[:], w_sb[i][:, which * fl:(which + 1) * fl],
                            x_t[:, i * tb:(i + 1) * tb],
                            start=(i == 0), stop=(i == kd - 1))
                    nc.vector.tensor_copy(dst[:, ts0:ts0 + tb], ps[:])
                # v (token-major): out [128 tok, fl]
                for tt in range(dtiles):
                    gti = tbi * dtiles + tt
                    ps_v = psum.tile([128, fl], F32, name="ps_v", tag="mm512", bufs=2)
                    for i in range(kd):
                        nc.tensor.matmul(
                            ps_v[:], x_t[:, i * tb + tt * 128:i * tb + (tt + 1) * 128],
                            w_sb[i][:, 2 * fl:3 * fl],
                            start=(i == 0), stop=(i == kd - 1))
                    nc.vector.tensor_copy(
                        v_sb.rearrange("p (n h c) -> p n h c", h=hpc,
                                       c=hd + 1)[:, gti, :, 0:hd],
                        ps_v[:].rearrange("p (h c) -> p h c", c=hd))

            def attn_block(b, qb):
                tq0 = b * seq + qb * tb
                ntk = (qb + 1) * dtiles
                # y accumulators: 2 one-bank tiles, 4 groups (2 qt x 2 heads)
                # each; group (qt, h) at cols (qt%2)*130 + h*65, width 65
                # (64 y + 1 denom). All matmuls accumulate (start=False) onto
                # an explicit memset so group order is irrelevant.
                ya = [psum.tile([128, 512], F32, name=f"ya{t}", tag=f"yacc{t}",
                                bufs=1) for t in range(2)]
                nc.vector.memset(ya[0][:, 0:2 * 130], 0.0)
                nc.vector.memset(ya[1][:, 0:2 * 130], 0.0)
                for tki in range(ntk):
                    t0 = b * seq + tki * 128
                    gti = t0 // 128
                    m = tki - qb * dtiles
                    # diagonal tile m covers only tq columns >= 128*m
                    c0 = 128 * m if m > 0 else 0
                    # both heads' scores side by side in one 2-bank tile
                    ps_s = psum.tile([128, hpc * tb], F32, name="ps_s",
                                     tag="s2", bufs=2)
                    p_t = work.tile([128, hpc * tb], BF16, name="p_t",
                                    tag="pt", bufs=6)
                    for h in range(hpc):
                        hs = slice(h * hd, (h + 1) * hd)
                        nc.tensor.matmul(ps_s[:, h * tb + c0:(h + 1) * tb],
                                         k_sb[hs, t0:t0 + 128],
                                         q_sb[hs, tq0 + c0:tq0 + tb],
                                         start=True, stop=True)
                    # one exp for both heads: AP [128, hpc, nq]
                    sv = ps_s[:].rearrange("p (h q) -> p h q", h=hpc)[:, :, c0:tb]
                    pv = p_t[:].rearrange("p (h q) -> p h q", h=hpc)[:, :, c0:tb]
                    nc.scalar.activation(pv, sv, AF.Exp, scale=scale)
                    if m >= 0:
                        # mask the [128,128] triangle at cols [c0, c0+128)
                        # (on DVE: Pool-engine work queues behind the 28us
                        # collectives and the monotonic Pool semaphore then
                        # stalls the PV Ldweights until the collective ends)
                        for h in range(hpc):
                            ap = p_t[:, h * tb + c0:h * tb + c0 + 128]
                            nc.vector.tensor_mul(ap, ap, tri[:])
                    for h in range(hpc):
                        vs = v_sb[:, gti * vw + h * (hd + 1):
                                  gti * vw + (h + 1) * (hd + 1)]
                        for qt in range(max(0, m), dtiles):
                            ti, g = divmod(qt, 2)
                            off = g * 130 + h * 65
                            nc.tensor.matmul(
                                ya[ti][:, off:off + 65],
                                p_t[:, h * tb + qt * 128:h * tb + (qt + 1) * 128],
                                vs, start=False,
                                stop=(tki == qb * dtiles + qt),
                                skip_group_check=True)
                # normalize: y_tok[:, gq, h*hd:...] = y/denom (token-major)
                for qt in range(dtiles):
                    ti, g = divmod(qt, 2)
                    gq = b * qpb + qb * dtiles + qt
                    for h in range(hpc):
                        off = g * 130 + h * 65
                        rcp = work.tile([128, 1], F32, name="rcp", tag="rcp",
                                        bufs=8)
                        nc.vector.reciprocal(rcp[:], ya[ti][:, off + 64:off + 65])
                        nc.vector.tensor_scalar_mul(
                            y_tok[:, gq * fl + h * hd:gq * fl + (h + 1) * hd],
                            ya[ti][:, off:off + 64], rcp[:])

            def a2a_issue(b):
                # shard j of batch b = qtiles b*qpb + [2j, 2j+2)
                for j in range(n_cores):
                    g0 = b * qpb + 2 * j
                    nc.gpsimd.dma_start(
                        a2a_in[b][j],
                        y_tok[:, g0 * fl:(g0 + 2) * fl])
                nc.gpsimd.collective_compute(
                    "AllToAll", mybir.AluOpType.bypass,
                    replica_groups=[list(range(n_cores))],
                    ins=[a2a_in[b][:]], outs=[a2a_out[b][:]],
                )

            def a2a_issue_half(b, hf):
                # shard j of half hf = qtile b*qpb + hf*8 + j
                for j in range(n_cores):
                    g0 = b * qpb + hf * (qpb // 2) + j
                    nc.gpsimd.dma_start(
                        a2ah_in[hf][j],
                        y_tok[:, g0 * fl:(g0 + 1) * fl])
                nc.gpsimd.collective_compute(
                    "AllToAll", mybir.AluOpType.bypass,
                    replica_groups=[list(range(n_cores))],
                    ins=[a2ah_in[hf][:]], outs=[a2ah_out[hf][:]],
                )

            def proj_half(pb, hf):
                # output cols [pb*tsb + hf*tsh, pb*tsb + (hf+1)*tsh)
                y_loc = [work.tile([fl, tsh], BF16, name="y_loch",
                                   tag=f"ylh{i}", bufs=1) for i in range(n_cores)]
                for i in range(n_cores):
                    nc.sync.dma_start_transpose(y_loc[i][:], a2ah_out[hf][i])
                c0o = pb * tsb + hf * tsh
                for dj in range(d // 128):
                    ps_o = psum.tile([128, tsh], F32, name="ps_oh", tag="mm512", bufs=2)
                    for i in range(n_cores):
                        nc.tensor.matmul(
                            ps_o[:], wp_sb[i][:, dj * 128:(dj + 1) * 128],
                            y_loc[i][:], start=(i == 0), stop=(i == n_cores - 1))
                    o_sb = work.tile([128, tsh], F32, name="o_sbh", tag="osb", bufs=3)
                    nc.vector.tensor_copy(o_sb[:], ps_o[:])
                    nc.sync.dma_start(
                        out[dj * 128:(dj + 1) * 128, c0o:c0o + tsh], o_sb[:])

            def proj(pb):
                y_loc = [work.tile([fl, tsb], BF16, name="y_loc",
                                   tag=f"yloc{i}", bufs=2) for i in range(n_cores)]
                for i in range(n_cores):
                    for t in range(2):
                        nc.sync.dma_start_transpose(
                            y_loc[i][:, t * 128:(t + 1) * 128],
                            a2a_out[pb][i][:, t * 128:(t + 1) * 128])
                for dj in range(d // 128):
                    ps_o = psum.tile([128, tsb], F32, name="ps_o", tag="mm512", bufs=2)
                    for i in range(n_cores):
                        nc.tensor.matmul(
                            ps_o[:], wp_sb[i][:, dj * 128:(dj + 1) * 128],
                            y_loc[i][:], start=(i == 0), stop=(i == n_cores - 1))
                    o_sb = work.tile([128, tsb], F32, name="o_sb", tag="osb", bufs=3)
                    nc.vector.tensor_copy(o_sb[:], ps_o[:])
                    nc.sync.dma_start(
                        out[dj * 128:(dj + 1) * 128, pb * tsb:(pb + 1) * tsb],
                        o_sb[:])

            units = [(b, qb) for b in range(bsz) for qb in range(nqb)]
            qkv_block(*units[0])
            if len(units) > 1:
                qkv_block(*units[1])
            for i in range(n_cores):
                nc.sync.dma_start(wp_sb[i][:], wproj[i * fl:(i + 1) * fl, :])
            last_b = bsz - 1
            for L, (b, qb) in enumerate(units):
                attn_block(b, qb)
                if L + 2 < len(units):
                    qkv_block(*units[L + 2])
                if split_last and b == last_b and qb == nqb // 2 - 1:
                    a2a_issue_half(b, 0)
                if qb == nqb - 1:
                    if split_last and b == last_b:
                        a2a_issue_half(b, 1)
                    else:
                        a2a_issue(b)
                    if b >= 1:
                        proj(b - 1)
            if split_last:
                proj_half(last_b, 0)
                proj_half(last_b, 1)
            else:
                proj(bsz - 1)
    _split_multi_waits(nc)
    return nc


def shard_inputs(x, w_qkv, w_proj, n_cores=N_CORES, n_head=N_HEAD):
    bf16 = ml_dtypes.bfloat16
    d = x.shape[-1]
    T = x.shape[0] * x.shape[1]
    hpc = n_head // n_cores
    fl = hpc * HD
    xT = np.ascontiguousarray(np.asarray(x, np.float32).reshape(T, d).T.astype(bf16))
    wq = np.asarray(w_qkv, np.float32)
    wp = np.ascontiguousarray(np.asarray(w_proj, np.float32).T.astype(bf16))
    in_maps = []
    for c in range(n_cores):
        r0 = c * fl
        wqkv_c = np.ascontiguousarray(
            np.concatenate([wq[r0:r0 + fl], wq[d + r0:d + r0 + fl],
                            wq[2 * d + r0:2 * d + r0 + fl]], axis=0).T.astype(bf16))
        in_maps.append({"xT": xT, "wqkv": wqkv_c, "wproj": wp})
    return in_maps


def assemble_out(outs, n_cores=N_CORES, bsz=BSZ, seq=SEQ, d=D):
    """outs[c] is [d, bsz*tsb]; column block b holds tokens
    b*seq + [c*tsb, (c+1)*tsb) — except the last batch when nqb==4, which is
    exchanged in halves: tokens b*seq + h*seq/2 + [c*tsh, (c+1)*tsh)."""
    tsb = seq // n_cores
    tsh = tsb // 2
    split_last = (seq // min(512, seq)) == 4
    T = bsz * seq
    outT = np.empty((d, T), np.float32)
    for c in range(n_cores):
        for b in range(bsz):
            if split_last and b == bsz - 1:
                for h in range(2):
                    base = b * seq + h * (seq // 2)
                    outT[:, base + c * tsh:base + (c + 1) * tsh] = \
                        outs[c][:, b * tsb + h * tsh:b * tsb + (h + 1) * tsh]
            else:
                outT[:, b * seq + c * tsb:b * seq + (c + 1) * tsb] = \
                    outs[c][:, b * tsb:(b + 1) * tsb]
    return np.ascontiguousarray(outT.T).reshape(bsz, seq, d)


_NC_CACHE = {}


def kernel(x, w_qkv, w_proj):
    key = "full"
    if key not in _NC_CACHE:
        _NC_CACHE[key] = build_nc()
    nc = _NC_CACHE[key]
    in_maps = shard_inputs(x, w_qkv, w_proj)
    res = run_bass_kernel_spmd(nc, in_maps, list(range(N_CORES))).results
    return assemble_out([res[c]["out"] for c in range(N_CORES)]).astype(np.float32)


# revision 11
# speedup vs baseline: 1.1038x; 1.0012x over previous
"""Causal self-attention (dense transformer) on 8 TRN2 NeuronCores.

Sharding: heads+batch tensor-parallel. Each core c owns 2 heads (2c, 2c+1)
for all 4 batches:
  - QKV projection with the w_qkv row-slice for its heads (x is replicated,
    fed pre-transposed as xT [d, tokens] so d lands on SBUF partitions).
  - Causal attention for its 8 (batch, head) pairs, computed in
    "transposed scores" layout S_t[tk, tq]. PV matmuls are token-major
    (stationary P^T tile, moving [V | 1]) so the output free dim is 65 wide
    instead of 512: y accumulates per 128-token q-tile as [tq, hd+1] with the
    softmax denominator in the last column.
  - Normalization is a per-partition reciprocal (DVE) + broadcast multiply
    (Pool) into a token-major y_tok buffer.
  - Per-batch AllToAll exchanges y token-major; the receiver DMA-transposes
    each [tok, fl] chunk back to feature-major y_loc for the output
    projection (last batch exchanges in two halves so its collective
    overlaps).
Host side: shard/transpose/cast inputs, concat+transpose the output.

Compute dtype bf16 (PSUM accumulation fp32), storage fp32 in/out.
"""

import numpy as np
import ml_dtypes

import concourse.bass as bass
import concourse.mybir as mybir
import concourse.tile as tile
from concourse.bass_utils import run_bass_kernel_spmd

BF16 = mybir.dt.bfloat16
F32 = mybir.dt.float32
AF = mybir.ActivationFunctionType

# Full-size problem constants (hardcoded per harness contract)
N_CORES = 8
BSZ, SEQ, D, N_HEAD = 4, 2048, 1024, 16
HD = 64  # head dim


def _split_multi_waits(nc):
    """walrus on this build accepts at most ONE sync-wait command per
    instruction. Hoist extra waits onto standalone same-engine nops placed
    immediately before the instruction (queue order preserves semantics)."""
    edits = []
    for func in nc.m.functions:
        for bb in func.blocks:
            insts = bb.instructions
            for idx, ins in enumerate(insts):
                si = ins.sync_info
                if si is not None and len(si.on_wait) > 1:
                    edits.append((bb, idx, ins))
    for bb, idx, ins in reversed(edits):
        si = ins.sync_info
        extra, keep = list(si.on_wait[:-1]), [si.on_wait[-1]]
        ins.sync_info = mybir.SyncInfo(on_wait=keep, on_update=list(si.on_update))
        nops = []
        for w in extra:
            nop = nc.engines[ins.engine].nop().ins
            host = nc.cur_bb.bb.instructions
            assert host[-1] is nop
            host.pop()
            nop.sync_info = mybir.SyncInfo(on_wait=[w], on_update=[])
            nops.append(nop)
        live = bb.instructions
        for j, nop in enumerate(nops):
            live.insert(idx + j, nop)


def build_nc(n_cores=N_CORES, bsz=BSZ, seq=SEQ, d=D, n_head=N_HEAD):
    hd = HD
    hpc = n_head // n_cores          # heads per core
    fl = hpc * hd                    # local feature width (q/k/v per core)
    T = bsz * seq                    # total tokens
    kd = d // 128                    # contraction tiles over d
    tb = min(512, seq)               # tq block width (matmul free dim)
    nqb = seq // tb                  # q-blocks per batch
    dtiles = tb // 128               # 128-tiles per q-block
    nt = T // 128                    # total 128-token tiles
    scale = float(1.0 / np.sqrt(hd))

    tsb = seq // n_cores             # per-batch token chunk per core
    qpb = seq // 128                 # 128-token q-tiles per batch
    nc = bass.Bass(num_devices=n_cores)
    xT = nc.declare_dram_parameter("xT", [d, T], BF16, isOutput=False)
    wqkv = nc.declare_dram_parameter("wqkv", [d, 3 * fl], BF16, isOutput=False)
    wproj = nc.declare_dram_parameter("wproj", [d, d], BF16, isOutput=False)
    out = nc.declare_dram_parameter("out", [d, bsz * tsb], F32, isOutput=True)
    split_last = (nqb == 4)          # last batch exchanges in 2 halves
    tsh = tsb // 2
    a2a_in = [nc.dram_tensor(f"a2a_in{b}", [n_cores, 128, tsb], BF16)
              for b in range(bsz - 1 if split_last else bsz)]
    a2a_out = [nc.dram_tensor(f"a2a_out{b}", [n_cores, 128, tsb], BF16)
               for b in range(bsz - 1 if split_last else bsz)]
    if split_last:
        a2ah_in = [nc.dram_tensor(f"a2ah_in{h}", [n_cores, 128, tsh], BF16)
                   for h in range(2)]
        a2ah_out = [nc.dram_tensor(f"a2ah_out{h}", [n_cores, 128, tsh], BF16)
                    for h in range(2)]

    with tile.TileContext(nc) as tc:
        with (
            tc.tile_pool(name="const", bufs=1) as const,
            tc.tile_pool(name="xin", bufs=2) as xin,
            tc.tile_pool(name="work", bufs=3) as work,
            tc.tile_pool(name="psum", bufs=1, space="PSUM") as psum,
        ):
            # ---- persistent SBUF ----
            w_sb = [const.tile([128, 3 * fl], BF16, name=f"w_sb{i}") for i in range(kd)]
            wp_sb = [const.tile([fl, d], BF16, name=f"wp_sb{i}") for i in range(n_cores)]

            q_sb = const.tile([fl, T], BF16, name="q_sb")
            k_sb = const.tile([fl, T], BF16, name="k_sb")
            # token-major y: [tok128, (global qtile, fl)]
            y_tok = const.tile([128, nt * fl], BF16, name="y_tok")
            # v token-major with a ones column per (tile, head): tile g, head h
            # occupies cols g*hpc*(hd+1) + h*(hd+1) + [0, hd+1); col hd is ones.
            vw = hpc * (hd + 1)
            v_sb = const.tile([128, nt * vw], BF16, name="v_sb")
            ones_ap = v_sb.rearrange("p (n h c) -> p n h c", h=hpc,
                                     c=hd + 1)[:, :, :, hd:hd + 1]
            nc.vector.memset(ones_ap, 1.0)

            # triangular mask [128,128]: keep S_t[tk_i, tq_j] iff i <= j
            tri = const.tile([128, 128], BF16, name="tri")
            nc.gpsimd.memset(tri[:], 1.0)
            nc.gpsimd.affine_select(
                out=tri[:], in_=tri[:],
                compare_op=mybir.AluOpType.is_ge, fill=0.0,
                base=0, channel_multiplier=-1, pattern=[[1, 128]],
            )

            # ---- unified (batch, q-block) stream with qkv prefetch ----
            def qkv_block(b, qb, x_pre=None):
                tbi = b * (seq // tb) + qb
                ts0 = tbi * tb
                if x_pre is None:
                    x_t = xin.tile([128, kd * tb], BF16, name="x_t", tag="x", bufs=3)
                    # one DMA: in [kd, 128, tb] (d-tiles) -> out [128, kd, tb]
                    nc.sync.dma_start(
                        x_t[:].rearrange("p (i t) -> p i t", i=kd),
                        xT.rearrange("(i p) T -> p i T", p=128)[:, :, ts0:ts0 + tb])
                else:
                    x_t = x_pre
                # q, k (feature-major): out [fl, tb]
                for which, dst in ((0, q_sb), (1, k_sb)):
                    ps = psum.tile([fl, tb], F32, name=f"ps_qk{which}", tag="mm512", bufs=2)
                    for i in range(kd):
                        nc.tensor.matmul(
                            ps[:], w_sb[i][:, which * fl:(which + 1) * fl],
                            x_t[:, i * tb:(i + 1) * tb],
                            start=(i == 0), stop=(i == kd - 1))
                    nc.vector.tensor_copy(dst[:, ts0:ts0 + tb], ps[:])
                # v (token-major): out [128 tok, fl]
                for tt in range(dtiles):
                    gti = tbi * dtiles + tt
                    ps_v = psum.tile([128, fl], F32, name="ps_v", tag="mm512", bufs=2)
                    for i in range(kd):
                        nc.tensor.matmul(
                            ps_v[:], x_t[:, i * tb + tt * 128:i * tb + (tt + 1) * 128],
                            w_sb[i][:, 2 * fl:3 * fl],
                            start=(i == 0), stop=(i == kd - 1))
                    nc.vector.tensor_copy(
                        v_sb.rearrange("p (n h c) -> p n h c", h=hpc,
                                       c=hd + 1)[:, gti, :, 0:hd],
                        ps_v[:].rearrange("p (h c) -> p h c", c=hd))

            def attn_block(b, qb):
                tq0 = b * seq + qb * tb
                ntk = (qb + 1) * dtiles
                # y accumulators: 2 one-bank tiles, 4 groups (2 qt x 2 heads)
                # each; group (qt, h) at cols (qt%2)*130 + h*65, width 65
                # (64 y + 1 denom). All matmuls accumulate (start=False) onto
                # an explicit memset so group order is irrelevant.
                ya = [psum.tile([128, 512], F32, name=f"ya{t}", tag=f"yacc{t}",
                                bufs=1) for t in range(2)]
                nc.vector.memset(ya[0][:, 0:2 * 130], 0.0)
                nc.vector.memset(ya[1][:, 0:2 * 130], 0.0)
                for tki in range(ntk):
                    t0 = b * seq + tki * 128
                    gti = t0 // 128
                    m = tki - qb * dtiles
                    # diagonal tile m covers only tq columns >= 128*m
                    c0 = 128 * m if m > 0 else 0
                    # both heads' scores side by side in one 2-bank tile
                    ps_s = psum.tile([128, hpc * tb], F32, name="ps_s",
                                     tag="s2", bufs=2)
                    p_t = work.tile([128, hpc * tb], BF16, name="p_t",
                                    tag="pt", bufs=6)
                    for h in range(hpc):
                        hs = slice(h * hd, (h + 1) * hd)
                        nc.tensor.matmul(ps_s[:, h * tb + c0:(h + 1) * tb],
                                         k_sb[hs, t0:t0 + 128],
                                         q_sb[hs, tq0 + c0:tq0 + tb],
                                         start=True, stop=True)
                    # one exp for both heads: AP [128, hpc, nq]
                    sv = ps_s[:].rearrange("p (h q) -> p h q", h=hpc)[:, :, c0:tb]
                    pv = p_t[:].rearrange("p (h q) -> p h q", h=hpc)[:, :, c0:tb]
                    nc.scalar.activation(pv, sv, AF.Exp, scale=scale)
                    if m >= 0:
                        # mask the [128,128] triangle at cols [c0, c0+128)
                        # (on DVE: Pool-engine work queues behind the 28us
                        # collectives and the monotonic Pool semaphore then
                        # stalls the PV Ldweights until the collective ends)
                        for h in range(hpc):
                            ap = p_t[:, h * tb + c0:h * tb + c0 + 128]
                            nc.vector.tensor_mul(ap, ap, tri[:])
                    for h in range(hpc):
                        vs = v_sb[:, gti * vw + h * (hd + 1):
                                  gti * vw + (h + 1) * (hd + 1)]
                        for qt in range(max(0, m), dtiles):
                            ti, g = divmod(qt, 2)
                            off = g * 130 + h * 65
                            nc.tensor.matmul(
                                ya[ti][:, off:off + 65],
                                p_t[:, h * tb + qt * 128:h * tb + (qt + 1) * 128],
                                vs, start=False,
                                stop=(tki == qb * dtiles + qt),
                                skip_group_check=True)
                # normalize: y_tok[:, gq, h*hd:...] = y/denom (token-major)
                for qt in range(dtiles):
                    ti, g = divmod(qt, 2)
                    gq = b * qpb + qb * dtiles + qt
                    for h in range(hpc):
                        off = g * 130 + h * 65
                        rcp = work.tile([128, 1], F32, name="rcp", tag="rcp",
                                        bufs=8)
                        nc.vector.reciprocal(rcp[:], ya[ti][:, off + 64:off + 65])
                        nc.vector.tensor_scalar_mul(
                            y_tok[:, gq * fl + h * hd:gq * fl + (h + 1) * hd],
                            ya[ti][:, off:off + 64], rcp[:])

            def a2a_issue(b):
                # shard j of batch b = qtiles b*qpb + [2j, 2j+2)
                for j in range(n_cores):
                    g0 = b * qpb + 2 * j
                    nc.gpsimd.dma_start(
                        a2a_in[b][j],
                        y_tok[:, g0 * fl:(g0 + 2) * fl])
                nc.gpsimd.collective_compute(
                    "AllToAll", mybir.AluOpType.bypass,
                    replica_groups=[list(range(n_cores))],
                    ins=[a2a_in[b][:]], outs=[a2a_out[b][:]],
                )

            def a2a_issue_half(b, hf):
                # shard j of half hf = qtile b*qpb + hf*8 + j
                for j in range(n_cores):
                    g0 = b * qpb + hf * (qpb // 2) + j
                    nc.gpsimd.dma_start(
                        a2ah_in[hf][j],
                        y_tok[:, g0 * fl:(g0 + 1) * fl])
                nc.gpsimd.collective_compute(
                    "AllToAll", mybir.AluOpType.bypass,
                    replica_groups=[list(range(n_cores))],
                    ins=[a2ah_in[hf][:]], outs=[a2ah_out[hf][:]],
                )

            def proj_half(pb, hf, not_before_ms):
                # output cols [pb*tsb + hf*tsh, pb*tsb + (hf+1)*tsh)
                with tc.tile_wait_until(not_before_ms):
                    y_loc = [work.tile([fl, tsh], BF16, name="y_loch",
                                       tag=f"ylh{i}", bufs=1) for i in range(n_cores)]
                    for i in range(n_cores):
                        nc.sync.dma_start_transpose(y_loc[i][:], a2ah_out[hf][i])
                    c0o = pb * tsb + hf * tsh
                    for dj in range(d // 128):
                        ps_o = psum.tile([128, tsh], F32, name="ps_oh", tag="mm512", bufs=2)
                        for i in range(n_cores):
                            nc.tensor.matmul(
                                ps_o[:], wp_sb[i][:, dj * 128:(dj + 1) * 128],
                                y_loc[i][:], start=(i == 0), stop=(i == n_cores - 1))
                        o_sb = work.tile([128, tsh], F32, name="o_sbh", tag="osb", bufs=3)
                        nc.scalar.copy(o_sb[:], ps_o[:])
                        nc.sync.dma_start(
                            out[dj * 128:(dj + 1) * 128, c0o:c0o + tsh], o_sb[:])

            def proj(pb, not_before_ms):
                with tc.tile_wait_until(not_before_ms):
                    y_loc = [work.tile([fl, tsb], BF16, name="y_loc",
                                       tag=f"yloc{i}", bufs=2) for i in range(n_cores)]
                    for i in range(n_cores):
                        for t in range(2):
                            nc.sync.dma_start_transpose(
                                y_loc[i][:, t * 128:(t + 1) * 128],
                                a2a_out[pb][i][:, t * 128:(t + 1) * 128])
                    for dj in range(d // 128):
                        ps_o = psum.tile([128, tsb], F32, name="ps_o", tag="mm512", bufs=2)
                        for i in range(n_cores):
                            nc.tensor.matmul(
                                ps_o[:], wp_sb[i][:, dj * 128:(dj + 1) * 128],
                                y_loc[i][:], start=(i == 0), stop=(i == n_cores - 1))
                        o_sb = work.tile([128, tsb], F32, name="o_sb", tag="osb", bufs=3)
                        nc.scalar.copy(o_sb[:], ps_o[:])
                        nc.sync.dma_start(
                            out[dj * 128:(dj + 1) * 128, pb * tsb:(pb + 1) * tsb],
                            o_sb[:])

            units = [(b, qb) for b in range(bsz) for qb in range(nqb)]
            # startup: first w chunk, then unit-0 x (its first matmuls need
            # only w_sb[0]), then the rest of w while matmuls proceed
            nc.sync.dma_start(w_sb[0][:], wqkv[0:128, :])
            ts00 = 0
            x_t0 = xin.tile([128, kd * tb], BF16, name="x_t", tag="x", bufs=3)
            nc.sync.dma_start(
                x_t0[:].rearrange("p (i t) -> p i t", i=kd),
                xT.rearrange("(i p) T -> p i T", p=128)[:, :, ts00:ts00 + tb])
            for i in range(1, kd):
                nc.sync.dma_start(w_sb[i][:], wqkv[i * 128:(i + 1) * 128, :])
            qkv_block(*units[0], x_pre=x_t0)
            if len(units) > 1:
                qkv_block(*units[1])
            for i in range(n_cores):
                nc.sync.dma_start(wp_sb[i][:], wproj[i * fl:(i + 1) * fl, :])
            last_b = bsz - 1
            # schedule hints: don't let the list scheduler hoist proj work
            # (which waits on its batch's collective) into the attention
            # stream before the collective can have finished.
            proj_ms = {0: 0.105, 1: 0.148, 2: 0.198}
            for L, (b, qb) in enumerate(units):
                attn_block(b, qb)
                if L + 2 < len(units):
                    qkv_block(*units[L + 2])
                if split_last and b == last_b and qb == nqb // 2 - 1:
                    a2a_issue_half(b, 0)
                if qb == nqb - 1:
                    if split_last and b == last_b:
                        a2a_issue_half(b, 1)
                    else:
                        a2a_issue(b)
                    if b >= 1:
                        proj(b - 1, proj_ms[b - 1])
            if split_last:
                proj_half(last_b, 0, 0.238)
                proj_half(last_b, 1, 0.272)
            else:
                proj(bsz - 1, 0.272)
    _split_multi_waits(nc)
    return nc


def shard_inputs(x, w_qkv, w_proj, n_cores=N_CORES, n_head=N_HEAD):
    bf16 = ml_dtypes.bfloat16
    d = x.shape[-1]
    T = x.shape[0] * x.shape[1]
    hpc = n_head // n_cores
    fl = hpc * HD
    xT = np.ascontiguousarray(np.asarray(x, np.float32).reshape(T, d).T.astype(bf16))
    wq = np.asarray(w_qkv, np.float32)
    wp = np.ascontiguousarray(np.asarray(w_proj, np.float32).T.astype(bf16))
    in_maps = []
    for c in range(n_cores):
        r0 = c * fl
        wqkv_c = np.ascontiguousarray(
            np.concatenate([wq[r0:r0 + fl], wq[d + r0:d + r0 + fl],
                            wq[2 * d + r0:2 * d + r0 + fl]], axis=0).T.astype(bf16))
        in_maps.append({"xT": xT, "wqkv": wqkv_c, "wproj": wp})
    return in_maps


def assemble_out(outs, n_cores=N_CORES, bsz=BSZ, seq=SEQ, d=D):
    """outs[c] is [d, bsz*tsb]; column block b holds tokens
    b*seq + [c*tsb, (c+1)*tsb) — except the last batch when nqb==4, which is
    exchanged in halves: tokens b*seq + h*seq/2 + [c*tsh, (c+1)*tsh)."""
    tsb = seq // n_cores
    tsh = tsb // 2
    split_last = (seq // min(512, seq)) == 4
    T = bsz * seq
    outT = np.empty((d, T), np.float32)
    for c in range(n_cores):
        for b in range(bsz):
            if split_last and b == bsz - 1:
                for h in range(2):
                    base = b * seq + h * (seq // 2)
                    outT[:, base + c * tsh:base + (c + 1) * tsh] = \
                        outs[c][:, b * tsb + h * tsh:b * tsb + (h + 1) * tsh]
            else:
                outT[:, b * seq + c * tsb:b * seq + (c + 1) * tsb] = \
                    outs[c][:, b * tsb:(b + 1) * tsb]
    return np.ascontiguousarray(outT.T).reshape(bsz, seq, d)


_NC_CACHE = {}


def kernel(x, w_qkv, w_proj):
    key = "full"
    if key not in _NC_CACHE:
        _NC_CACHE[key] = build_nc()
    nc = _NC_CACHE[key]
    in_maps = shard_inputs(x, w_qkv, w_proj)
    res = run_bass_kernel_spmd(nc, in_maps, list(range(N_CORES))).results
    return assemble_out([res[c]["out"] for c in range(N_CORES)]).astype(np.float32)


# revision 13
# speedup vs baseline: 1.1450x; 1.0373x over previous
"""Causal self-attention (dense transformer) on 8 TRN2 NeuronCores.

Sharding: heads+batch tensor-parallel. Each core c owns 2 heads (2c, 2c+1)
for all 4 batches:
  - QKV projection with the w_qkv row-slice for its heads (x is replicated,
    fed pre-transposed as xT [d, tokens] so d lands on SBUF partitions).
  - Causal attention for its 8 (batch, head) pairs, computed in
    "transposed scores" layout S_t[tk, tq]. PV matmuls are token-major
    (stationary P^T tile, moving [V | 1]) so the output free dim is 65 wide
    instead of 512: y accumulates per 128-token q-tile as [tq, hd+1] with the
    softmax denominator in the last column.
  - Normalization is a per-partition reciprocal (DVE) + broadcast multiply
    (Pool) into a token-major y_tok buffer.
  - Per-batch AllToAll exchanges y token-major; the receiver DMA-transposes
    each [tok, fl] chunk back to feature-major y_loc for the output
    projection (last batch exchanges in two halves so its collective
    overlaps).
Host side: shard/transpose/cast inputs, concat+transpose the output.

Compute dtype bf16 (PSUM accumulation fp32), storage fp32 in/out.
"""

import numpy as np
import ml_dtypes

import concourse.bass as bass
import concourse.mybir as mybir
import concourse.tile as tile
from concourse.bass_utils import run_bass_kernel_spmd

BF16 = mybir.dt.bfloat16
F32 = mybir.dt.float32
AF = mybir.ActivationFunctionType

# Full-size problem constants (hardcoded per harness contract)
N_CORES = 8
BSZ, SEQ, D, N_HEAD = 4, 2048, 1024, 16
HD = 64  # head dim


def _split_multi_waits(nc):
    """walrus on this build accepts at most ONE sync-wait command per
    instruction. Hoist extra waits onto standalone same-engine nops placed
    immediately before the instruction (queue order preserves semantics)."""
    edits = []
    for func in nc.m.functions:
        for bb in func.blocks:
            insts = bb.instructions
            for idx, ins in enumerate(insts):
                si = ins.sync_info
                if si is not None and len(si.on_wait) > 1:
                    edits.append((bb, idx, ins))
    for bb, idx, ins in reversed(edits):
        si = ins.sync_info
        extra, keep = list(si.on_wait[:-1]), [si.on_wait[-1]]
        ins.sync_info = mybir.SyncInfo(on_wait=keep, on_update=list(si.on_update))
        nops = []
        for w in extra:
            nop = nc.engines[ins.engine].nop().ins
            host = nc.cur_bb.bb.instructions
            assert host[-1] is nop
            host.pop()
            nop.sync_info = mybir.SyncInfo(on_wait=[w], on_update=[])
            nops.append(nop)
        live = bb.instructions
        for j, nop in enumerate(nops):
            live.insert(idx + j, nop)


def build_nc(n_cores=N_CORES, bsz=BSZ, seq=SEQ, d=D, n_head=N_HEAD):
    hd = HD
    hpc = n_head // n_cores          # heads per core
    fl = hpc * hd                    # local feature width (q/k/v per core)
    T = bsz * seq                    # total tokens
    kd = d // 128                    # contraction tiles over d
    tb = min(512, seq)               # tq block width (matmul free dim)
    nqb = seq // tb                  # q-blocks per batch
    dtiles = tb // 128               # 128-tiles per q-block
    nt = T // 128                    # total 128-token tiles
    scale = float(1.0 / np.sqrt(hd))

    tsb = seq // n_cores             # per-batch token chunk per core
    qpb = seq // 128                 # 128-token q-tiles per batch
    nc = bass.Bass(num_devices=n_cores)
    xT = nc.declare_dram_parameter("xT", [d, T], BF16, isOutput=False)
    wqkv = nc.declare_dram_parameter("wqkv", [d, 3 * fl], BF16, isOutput=False)
    wproj = nc.declare_dram_parameter("wproj", [d, d], BF16, isOutput=False)
    out = nc.declare_dram_parameter("out", [d, bsz * tsb], F32, isOutput=True)
    split_last = (nqb == 4)          # last batch exchanges in 2 halves
    tsh = tsb // 2
    a2a_in = [nc.dram_tensor(f"a2a_in{b}", [n_cores, 128, tsb], BF16)
              for b in range(bsz - 1 if split_last else bsz)]
    a2a_out = [nc.dram_tensor(f"a2a_out{b}", [n_cores, 128, tsb], BF16)
               for b in range(bsz - 1 if split_last else bsz)]
    if split_last:
        a2ah_in = [nc.dram_tensor(f"a2ah_in{h}", [n_cores, 128, tsh], BF16)
                   for h in range(2)]
        a2ah_out = [nc.dram_tensor(f"a2ah_out{h}", [n_cores, 128, tsh], BF16)
                    for h in range(2)]

    with tile.TileContext(nc) as tc:
        with (
            tc.tile_pool(name="const", bufs=1) as const,
            tc.tile_pool(name="xin", bufs=2) as xin,
            tc.tile_pool(name="work", bufs=3) as work,
            tc.tile_pool(name="psum", bufs=1, space="PSUM") as psum,
        ):
            # ---- persistent SBUF ----
            w_sb = [const.tile([128, 3 * fl], BF16, name=f"w_sb{i}") for i in range(kd)]
            wp_sb = [const.tile([fl, d], BF16, name=f"wp_sb{i}") for i in range(n_cores)]

            q_sb = const.tile([fl, T], BF16, name="q_sb")
            k_sb = const.tile([fl, T], BF16, name="k_sb")
            # token-major y: [tok128, (global qtile, fl)]
            y_tok = const.tile([128, nt * fl], BF16, name="y_tok")
            # v token-major with a ones column per (tile, head): tile g, head h
            # occupies cols g*hpc*(hd+1) + h*(hd+1) + [0, hd+1); col hd is ones.
            vw = hpc * (hd + 1)
            v_sb = const.tile([128, nt * vw], BF16, name="v_sb")
            ones_ap = v_sb.rearrange("p (n h c) -> p n h c", h=hpc,
                                     c=hd + 1)[:, :, :, hd:hd + 1]
            nc.vector.memset(ones_ap, 1.0)

            # triangular mask [128,128]: keep S_t[tk_i, tq_j] iff i <= j
            tri = const.tile([128, 128], BF16, name="tri")
            nc.gpsimd.memset(tri[:], 1.0)
            nc.gpsimd.affine_select(
                out=tri[:], in_=tri[:],
                compare_op=mybir.AluOpType.is_ge, fill=0.0,
                base=0, channel_multiplier=-1, pattern=[[1, 128]],
            )

            # ---- unified (batch, q-block) stream with qkv prefetch ----
            def qkv_block(b, qb, x_pre=None):
                tbi = b * (seq // tb) + qb
                ts0 = tbi * tb
                if x_pre is None:
                    x_t = xin.tile([128, kd * tb], BF16, name="x_t", tag="x", bufs=3)
                    # one DMA: in [kd, 128, tb] (d-tiles) -> out [128, kd, tb]
                    nc.sync.dma_start(
                        x_t[:].rearrange("p (i t) -> p i t", i=kd),
                        xT.rearrange("(i p) T -> p i T", p=128)[:, :, ts0:ts0 + tb])
                else:
                    x_t = x_pre
                # q, k (feature-major): out [fl, tb]
                for which, dst in ((0, q_sb), (1, k_sb)):
                    ps = psum.tile([fl, tb], F32, name=f"ps_qk{which}", tag="mm512", bufs=2)
                    for i in range(kd):
                        nc.tensor.matmul(
                            ps[:], w_sb[i][:, which * fl:(which + 1) * fl],
                            x_t[:, i * tb:(i + 1) * tb],
                            start=(i == 0), stop=(i == kd - 1))
                    nc.vector.tensor_copy(dst[:, ts0:ts0 + tb], ps[:])
                # v (token-major): out [128 tok, fl]
                for tt in range(dtiles):
                    gti = tbi * dtiles + tt
                    ps_v = psum.tile([128, fl], F32, name="ps_v", tag="mm512", bufs=2)
                    for i in range(kd):
                        nc.tensor.matmul(
                            ps_v[:], x_t[:, i * tb + tt * 128:i * tb + (tt + 1) * 128],
                            w_sb[i][:, 2 * fl:3 * fl],
                            start=(i == 0), stop=(i == kd - 1))
                    nc.vector.tensor_copy(
                        v_sb.rearrange("p (n h c) -> p n h c", h=hpc,
                                       c=hd + 1)[:, gti, :, 0:hd],
                        ps_v[:].rearrange("p (h c) -> p h c", c=hd))

            def attn_block(b, qb):
                tq0 = b * seq + qb * tb
                ntk = (qb + 1) * dtiles
                # y accumulators: 2 one-bank tiles, 4 groups (2 qt x 2 heads)
                # each; group (qt, h) at cols (qt%2)*130 + h*65, width 65
                # (64 y + 1 denom). All matmuls accumulate (start=False) onto
                # an explicit memset so group order is irrelevant.
                ya = [psum.tile([128, 512], F32, name=f"ya{t}", tag=f"yacc{t}",
                                bufs=1) for t in range(2)]
                nc.vector.memset(ya[0][:, 0:2 * 130], 0.0)
                nc.vector.memset(ya[1][:, 0:2 * 130], 0.0)
                for tki in range(ntk):
                    t0 = b * seq + tki * 128
                    gti = t0 // 128
                    m = tki - qb * dtiles
                    # diagonal tile m covers only tq columns >= 128*m
                    c0 = 128 * m if m > 0 else 0
                    # both heads' scores side by side in one 2-bank tile
                    ps_s = psum.tile([128, hpc * tb], F32, name="ps_s",
                                     tag="s2", bufs=2)
                    p_t = work.tile([128, hpc * tb], BF16, name="p_t",
                                    tag="pt", bufs=6)
                    for h in range(hpc):
                        hs = slice(h * hd, (h + 1) * hd)
                        nc.tensor.matmul(ps_s[:, h * tb + c0:(h + 1) * tb],
                                         k_sb[hs, t0:t0 + 128],
                                         q_sb[hs, tq0 + c0:tq0 + tb],
                                         start=True, stop=True)
                    # one exp for both heads: AP [128, hpc, nq]
                    sv = ps_s[:].rearrange("p (h q) -> p h q", h=hpc)[:, :, c0:tb]
                    pv = p_t[:].rearrange("p (h q) -> p h q", h=hpc)[:, :, c0:tb]
                    nc.scalar.activation(pv, sv, AF.Exp, scale=scale)
                    if m >= 0:
                        # mask the [128,128] triangle at cols [c0, c0+128)
                        # (on DVE: Pool-engine work queues behind the 28us
                        # collectives and the monotonic Pool semaphore then
                        # stalls the PV Ldweights until the collective ends)
                        for h in range(hpc):
                            ap = p_t[:, h * tb + c0:h * tb + c0 + 128]
                            nc.vector.tensor_mul(ap, ap, tri[:])
                    for h in range(hpc):
                        vs = v_sb[:, gti * vw + h * (hd + 1):
                                  gti * vw + (h + 1) * (hd + 1)]
                        for qt in range(max(0, m), dtiles):
                            ti, g = divmod(qt, 2)
                            off = g * 130 + h * 65
                            nc.tensor.matmul(
                                ya[ti][:, off:off + 65],
                                p_t[:, h * tb + qt * 128:h * tb + (qt + 1) * 128],
                                vs, start=False,
                                stop=(tki == qb * dtiles + qt),
                                skip_group_check=True)
                # normalize: y_tok[:, gq, h*hd:...] = y/denom (token-major)
                for qt in range(dtiles):
                    ti, g = divmod(qt, 2)
                    gq = b * qpb + qb * dtiles + qt
                    for h in range(hpc):
                        off = g * 130 + h * 65
                        rcp = work.tile([128, 1], F32, name="rcp", tag="rcp",
                                        bufs=8)
                        nc.vector.reciprocal(rcp[:], ya[ti][:, off + 64:off + 65])
                        nc.vector.tensor_scalar_mul(
                            y_tok[:, gq * fl + h * hd:gq * fl + (h + 1) * hd],
                            ya[ti][:, off:off + 64], rcp[:])

            def a2a_issue(b):
                # shard j of batch b = qtiles b*qpb + [2j, 2j+2)
                for j in range(n_cores):
                    g0 = b * qpb + 2 * j
                    nc.gpsimd.dma_start(
                        a2a_in[b][j],
                        y_tok[:, g0 * fl:(g0 + 2) * fl])
                nc.gpsimd.collective_compute(
                    "AllToAll", mybir.AluOpType.bypass,
                    replica_groups=[list(range(n_cores))],
                    ins=[a2a_in[b][:]], outs=[a2a_out[b][:]],
                )

            def a2a_issue_half(b, hf):
                # shard j of half hf = qtile b*qpb + hf*8 + j
                for j in range(n_cores):
                    g0 = b * qpb + hf * (qpb // 2) + j
                    nc.gpsimd.dma_start(
                        a2ah_in[hf][j],
                        y_tok[:, g0 * fl:(g0 + 1) * fl])
                nc.gpsimd.collective_compute(
                    "AllToAll", mybir.AluOpType.bypass,
                    replica_groups=[list(range(n_cores))],
                    ins=[a2ah_in[hf][:]], outs=[a2ah_out[hf][:]],
                )

            def proj_half(pb, hf, not_before_ms):
                # output cols [pb*tsb + hf*tsh, pb*tsb + (hf+1)*tsh)
                with tc.tile_wait_until(not_before_ms):
                    y_loc = [work.tile([fl, tsh], BF16, name="y_loch",
                                       tag=f"ylh{i}", bufs=1) for i in range(n_cores)]
                    for i in range(n_cores):
                        nc.sync.dma_start_transpose(y_loc[i][:], a2ah_out[hf][i])
                    c0o = pb * tsb + hf * tsh
                    for dj in range(d // 128):
                        ps_o = psum.tile([128, tsh], F32, name="ps_oh", tag="mm512", bufs=2)
                        for i in range(n_cores):
                            nc.tensor.matmul(
                                ps_o[:], wp_sb[i][:, dj * 128:(dj + 1) * 128],
                                y_loc[i][:], start=(i == 0), stop=(i == n_cores - 1))
                        o_sb = work.tile([128, tsh], F32, name="o_sbh", tag="osb", bufs=3)
                        nc.scalar.copy(o_sb[:], ps_o[:])
                        nc.sync.dma_start(
                            out[dj * 128:(dj + 1) * 128, c0o:c0o + tsh], o_sb[:])

            def proj(pb, not_before_ms):
                with tc.tile_wait_until(not_before_ms):
                    y_loc = [work.tile([fl, tsb], BF16, name="y_loc",
                                       tag=f"yloc{i}", bufs=2) for i in range(n_cores)]
                    for i in range(n_cores):
                        for t in range(2):
                            nc.sync.dma_start_transpose(
                                y_loc[i][:, t * 128:(t + 1) * 128],
                                a2a_out[pb][i][:, t * 128:(t + 1) * 128])
                    for dj in range(d // 128):
                        ps_o = psum.tile([128, tsb], F32, name="ps_o", tag="mm512", bufs=2)
                        for i in range(n_cores):
                            nc.tensor.matmul(
                                ps_o[:], wp_sb[i][:, dj * 128:(dj + 1) * 128],
                                y_loc[i][:], start=(i == 0), stop=(i == n_cores - 1))
                        o_sb = work.tile([128, tsb], F32, name="o_sb", tag="osb", bufs=3)
                        nc.scalar.copy(o_sb[:], ps_o[:])
                        nc.sync.dma_start(
                            out[dj * 128:(dj + 1) * 128, pb * tsb:(pb + 1) * tsb],
                            o_sb[:])

            units = [(b, qb) for b in range(bsz) for qb in range(nqb)]
            # startup: first w chunk, then unit-0 x (its first matmuls need
            # only w_sb[0]), then the rest of w while matmuls proceed
            nc.sync.dma_start(w_sb[0][:], wqkv[0:128, :])
            ts00 = 0
            x_t0 = xin.tile([128, kd * tb], BF16, name="x_t", tag="x", bufs=3)
            nc.sync.dma_start(
                x_t0[:].rearrange("p (i t) -> p i t", i=kd),
                xT.rearrange("(i p) T -> p i T", p=128)[:, :, ts00:ts00 + tb])
            for i in range(1, kd):
                nc.sync.dma_start(w_sb[i][:], wqkv[i * 128:(i + 1) * 128, :])
            qkv_block(*units[0], x_pre=x_t0)
            if len(units) > 1:
                qkv_block(*units[1])
            for i in range(n_cores):
                nc.sync.dma_start(wp_sb[i][:], wproj[i * fl:(i + 1) * fl, :])
            last_b = bsz - 1
            # schedule hints: don't let the list scheduler hoist proj work
            # (which waits on its batch's collective) into the attention
            # stream before the collective can have finished.
            proj_ms = {0: 0.094, 1: 0.137, 2: 0.186}
            for L, (b, qb) in enumerate(units):
                attn_block(b, qb)
                if L + 2 < len(units):
                    qkv_block(*units[L + 2])
                if split_last and b == last_b and qb == nqb // 2 - 1:
                    a2a_issue_half(b, 0)
                if qb == nqb - 1:
                    if split_last and b == last_b:
                        a2a_issue_half(b, 1)
                    else:
                        a2a_issue(b)
                    if b >= 1:
                        proj(b - 1, proj_ms[b - 1])
            if split_last:
                proj_half(last_b, 0, 0.211)
                proj_half(last_b, 1, 0.2455)
            else:
                proj(bsz - 1, 0.2455)
    _split_multi_waits(nc)
    return nc


def shard_inputs(x, w_qkv, w_proj, n_cores=N_CORES, n_head=N_HEAD):
    bf16 = ml_dtypes.bfloat16
    d = x.shape[-1]
    T = x.shape[0] * x.shape[1]
    hpc = n_head // n_cores
    fl = hpc * HD
    xT = np.ascontiguousarray(np.asarray(x, np.float32).reshape(T, d).T.astype(bf16))
    wq = np.asarray(w_qkv, np.float32)
    wp = np.ascontiguousarray(np.asarray(w_proj, np.float32).T.astype(bf16))
    in_maps = []
    for c in range(n_cores):
        r0 = c * fl
        wqkv_c = np.ascontiguousarray(
            np.concatenate([wq[r0:r0 + fl], wq[d + r0:d + r0 + fl],
                            wq[2 * d + r0:2 * d + r0 + fl]], axis=0).T.astype(bf16))
        in_maps.append({"xT": xT, "wqkv": wqkv_c, "wproj": wp})
    return in_maps


def assemble_out(outs, n_cores=N_CORES, bsz=BSZ, seq=SEQ, d=D):
    """outs[c] is [d, bsz*tsb]; column block b holds tokens
    b*seq + [c*tsb, (c+1)*tsb) — except the last batch when nqb==4, which is
    exchanged in halves: tokens b*seq + h*seq/2 + [c*tsh, (c+1)*tsh)."""
    tsb = seq // n_cores
    tsh = tsb // 2
    split_last = (seq // min(512, seq)) == 4
    T = bsz * seq
    outT = np.empty((d, T), np.float32)
    for c in range(n_cores):
        for b in range(bsz):
            if split_last and b == bsz - 1:
                for h in range(2):
                    base = b * seq + h * (seq // 2)
                    outT[:, base + c * tsh:base + (c + 1) * tsh] = \
                        outs[c][:, b * tsb + h * tsh:b * tsb + (h + 1) * tsh]
            else:
                outT[:, b * seq + c * tsb:b * seq + (c + 1) * tsb] = \
                    outs[c][:, b * tsb:(b + 1) * tsb]
    return np.ascontiguousarray(outT.T).reshape(bsz, seq, d)


_NC_CACHE = {}


def kernel(x, w_qkv, w_proj):
    key = "full"
    if key not in _NC_CACHE:
        _NC_CACHE[key] = build_nc()
    nc = _NC_CACHE[key]
    in_maps = shard_inputs(x, w_qkv, w_proj)
    res = run_bass_kernel_spmd(nc, in_maps, list(range(N_CORES))).results
    return assemble_out([res[c]["out"] for c in range(N_CORES)]).astype(np.float32)


# revision 20
# speedup vs baseline: 1.2181x; 1.0639x over previous
"""Causal self-attention (dense transformer) on 8 TRN2 NeuronCores.

Sharding: heads+batch tensor-parallel. Each core c owns 2 heads (2c, 2c+1)
for all 4 batches:
  - QKV projection with the w_qkv row-slice for its heads (x is replicated,
    fed pre-transposed as xT [d, tokens] so d lands on SBUF partitions).
  - Causal attention for its 8 (batch, head) pairs, computed in
    "transposed scores" layout S_t[tk, tq]. PV matmuls are token-major
    (stationary P^T tile, moving [V | 1]) so the output free dim is 65 wide
    instead of 512: y accumulates per 128-token q-tile as [tq, hd+1] with the
    softmax denominator in the last column.
  - Normalization is a per-partition reciprocal (DVE) + broadcast multiply
    (Pool) into a token-major y_tok buffer.
  - Per-batch AllToAll exchanges y token-major; the receiver DMA-transposes
    each [tok, fl] chunk back to feature-major y_loc for the output
    projection (last batch exchanges in two halves so its collective
    overlaps).
Host side: shard/transpose/cast inputs, concat+transpose the output.

Compute dtype bf16 (PSUM accumulation fp32), storage fp32 in/out.
"""

import numpy as np
import ml_dtypes

import concourse.bass as bass
import concourse.mybir as mybir
import concourse.tile as tile
from concourse.bass_utils import run_bass_kernel_spmd

BF16 = mybir.dt.bfloat16
F32 = mybir.dt.float32
F8 = mybir.dt.float8e4
AF = mybir.ActivationFunctionType
DR = mybir.MatmulPerfMode.DoubleRow

# fp8 residual scales: x ~ (xh + xl)/SX, w ~ (wh + wl)/SW elementwise, with
# xh/xl/wh/wl e4m3 (max 240). qkv output scale = 1/(SX*SW).
SX = 16.0
SW = 256.0

# Full-size problem constants (hardcoded per harness contract)
N_CORES = 8
BSZ, SEQ, D, N_HEAD = 4, 2048, 1024, 16
HD = 64  # head dim


def _split_multi_waits(nc):
    """walrus on this build accepts at most ONE sync-wait command per
    instruction. Hoist extra waits onto standalone same-engine nops placed
    immediately before the instruction (queue order preserves semantics)."""
    edits = []
    for func in nc.m.functions:
        for bb in func.blocks:
            insts = bb.instructions
            for idx, ins in enumerate(insts):
                si = ins.sync_info
                if si is not None and len(si.on_wait) > 1:
                    edits.append((bb, idx, ins))
    for bb, idx, ins in reversed(edits):
        si = ins.sync_info
        extra, keep = list(si.on_wait[:-1]), [si.on_wait[-1]]
        ins.sync_info = mybir.SyncInfo(on_wait=keep, on_update=list(si.on_update))
        nops = []
        for w in extra:
            nop = nc.engines[ins.engine].nop().ins
            host = nc.cur_bb.bb.instructions
            assert host[-1] is nop
            host.pop()
            nop.sync_info = mybir.SyncInfo(on_wait=[w], on_update=[])
            nops.append(nop)
        live = bb.instructions
        for j, nop in enumerate(nops):
            live.insert(idx + j, nop)


def build_nc(n_cores=N_CORES, bsz=BSZ, seq=SEQ, d=D, n_head=N_HEAD):
    hd = HD
    hpc = n_head // n_cores          # heads per core
    fl = hpc * hd                    # local feature width (q/k/v per core)
    T = bsz * seq                    # total tokens
    kd = d // 128                    # contraction tiles over d
    tb = min(512, seq)               # tq block width (matmul free dim)
    nqb = seq // tb                  # q-blocks per batch
    dtiles = tb // 128               # 128-tiles per q-block
    nt = T // 128                    # total 128-token tiles
    scale = float(1.0 / np.sqrt(hd))

    tsb = seq // n_cores             # per-batch token chunk per core
    qpb = seq // 128                 # 128-token q-tiles per batch
    nc = bass.Bass(num_devices=n_cores)
    xh_d = nc.declare_dram_parameter("xh", [d, T], F8, isOutput=False)
    xl_d = nc.declare_dram_parameter("xl", [d, T], F8, isOutput=False)
    wqh_d = nc.declare_dram_parameter("wqh", [d, 3 * fl], F8, isOutput=False)
    wql_d = nc.declare_dram_parameter("wql", [d, 3 * fl], F8, isOutput=False)
    wproj = nc.declare_dram_parameter("wproj", [d, d], BF16, isOutput=False)
    out = nc.declare_dram_parameter("out", [d, bsz * tsb], F32, isOutput=True)
    split_last = (nqb == 4)          # last batch exchanges in 2 halves
    tsh = tsb // 2
    a2a_in = [nc.dram_tensor(f"a2a_in{b}", [n_cores, 128, tsb], BF16)
              for b in range(bsz - 1 if split_last else bsz)]
    a2a_out = [nc.dram_tensor(f"a2a_out{b}", [n_cores, 128, tsb], BF16)
               for b in range(bsz - 1 if split_last else bsz)]
    if split_last:
        a2ah_in = [nc.dram_tensor(f"a2ah_in{h}", [n_cores, 128, tsh], BF16)
                   for h in range(2)]
        a2ah_out = [nc.dram_tensor(f"a2ah_out{h}", [n_cores, 128, tsh], BF16)
                    for h in range(2)]

    with tile.TileContext(nc) as tc:
        with (
            tc.tile_pool(name="const", bufs=1) as const,
            tc.tile_pool(name="xin", bufs=2) as xin,
            tc.tile_pool(name="work", bufs=3) as work,
            tc.tile_pool(name="psum", bufs=1, space="PSUM") as psum,
        ):
            # ---- persistent SBUF ----
            # qkv weights as fp8 hi/lo, ktile-PAIR layout for DoubleRow:
            # w8[h][j] free dims (i, c): i = pair half (ktile 2j+i), c = col
            npair = kd // 2
            w_hi = [const.tile([128, 2 * 3 * fl], F8, name=f"w_hi{j}")
                    for j in range(npair)]
            w_lo = [const.tile([128, 2 * 3 * fl], F8, name=f"w_lo{j}")
                    for j in range(npair)]
            wp_sb = [const.tile([fl, d], BF16, name=f"wp_sb{i}") for i in range(n_cores)]

            q_sb = const.tile([fl, T], BF16, name="q_sb")
            k_sb = const.tile([fl, T], BF16, name="k_sb")
            # token-major y: [tok128, (global qtile, fl)]
            y_tok = const.tile([128, nt * fl], BF16, name="y_tok")
            # v token-major with a ones column per (tile, head): tile g, head h
            # occupies cols g*hpc*(hd+1) + h*(hd+1) + [0, hd+1); col hd is ones.
            vw = hpc * (hd + 1)
            v_sb = const.tile([128, nt * vw], BF16, name="v_sb")
            ones_ap = v_sb.rearrange("p (n h c) -> p n h c", h=hpc,
                                     c=hd + 1)[:, :, :, hd:hd + 1]
            nc.vector.memset(ones_ap, 1.0)

            # triangular mask [128,128]: keep S_t[tk_i, tq_j] iff i <= j
            tri = const.tile([128, 128], BF16, name="tri")
            nc.gpsimd.memset(tri[:], 1.0)
            nc.gpsimd.affine_select(
                out=tri[:], in_=tri[:],
                compare_op=mybir.AluOpType.is_ge, fill=0.0,
                base=0, channel_multiplier=-1, pattern=[[1, 128]],
            )

            # ---- unified (batch, q-block) stream with qkv prefetch ----
            def load_x8(ts0):
                x_hi = xin.tile([128, kd * tb], F8, name="x_hi", tag="xh", bufs=3)
                x_lo = xin.tile([128, kd * tb], F8, name="x_lo", tag="xl", bufs=3)
                # one DMA each: in [(j i p), T] -> out [128, j, i, tb]
                for t8, src in ((x_hi, xh_d), (x_lo, xl_d)):
                    nc.sync.dma_start(
                        t8[:].rearrange("p (j i t) -> p j i t", j=npair, i=2),
                        src.rearrange("(j i p) T -> p j i T", p=128,
                                      i=2)[:, :, :, ts0:ts0 + tb])
                return x_hi, x_lo

            def qkv_block(b, qb, x_pre=None):
                tbi = b * (seq // tb) + qb
                ts0 = tbi * tb
                if x_pre is None:
                    x_hi, x_lo = load_x8(ts0)
                else:
                    x_hi, x_lo = x_pre
                xh_r = x_hi[:].rearrange("p (j i t) -> p j i t", j=npair, i=2)
                xl_r = x_lo[:].rearrange("p (j i t) -> p j i t", j=npair, i=2)
                wh_r = [w_hi[j][:].rearrange("p (i c) -> p i c", i=2)
                        for j in range(npair)]
                wl_r = [w_lo[j][:].rearrange("p (i c) -> p i c", i=2)
                        for j in range(npair)]
                # q, k (feature-major): out [fl, tb], fp8 DoubleRow residual
                # (hi*hi + lo*hi + hi*lo; each instr contracts a ktile pair)
                for which, dst in ((0, q_sb), (1, k_sb)):
                    cs = slice(which * fl, (which + 1) * fl)
                    ps = psum.tile([fl, tb], F32, name=f"ps_qk{which}", tag="mm512", bufs=2)
                    n3 = 3 * npair
                    ii = 0
                    for j in range(npair):
                        for wop, xop in ((wh_r[j], xh_r), (wl_r[j], xh_r),
                                         (wh_r[j], xl_r)):
                            nc.tensor.matmul(
                                ps[:], wop[:, :, cs], xop[:, j],
                                start=(ii == 0), stop=(ii == n3 - 1),
                                perf_mode=DR)
                            ii += 1
                    nc.vector.tensor_scalar_mul(dst[:, ts0:ts0 + tb], ps[:],
                                                1.0 / (SX * SW))
                # v (token-major): out [128 tok, fl]
                vcs = slice(2 * fl, 3 * fl)
                for tt in range(dtiles):
                    gti = tbi * dtiles + tt
                    tts = slice(tt * 128, (tt + 1) * 128)
                    ps_v = psum.tile([128, fl], F32, name="ps_v", tag="mm512", bufs=2)
                    n3 = 3 * npair
                    ii = 0
                    for j in range(npair):
                        for xop, wop in ((xh_r, wh_r[j]), (xl_r, wh_r[j]),
                                         (xh_r, wl_r[j])):
                            nc.tensor.matmul(
                                ps_v[:], xop[:, j, :, tts], wop[:, :, vcs],
                                start=(ii == 0), stop=(ii == n3 - 1),
                                perf_mode=DR)
                            ii += 1
                    nc.vector.tensor_scalar_mul(
                        v_sb.rearrange("p (n h c) -> p n h c", h=hpc,
                                       c=hd + 1)[:, gti, :, 0:hd],
                        ps_v[:].rearrange("p (h c) -> p h c", c=hd),
                        1.0 / (SX * SW))

            def attn_block(b, qb):
                tq0 = b * seq + qb * tb
                ntk = (qb + 1) * dtiles
                # y accumulators: 2 one-bank tiles, 4 groups (2 qt x 2 heads)
                # each; group (qt, h) at cols (qt%2)*130 + h*65, width 65
                # (64 y + 1 denom). All matmuls accumulate (start=False) onto
                # an explicit memset so group order is irrelevant.
                ya = [psum.tile([128, 512], F32, name=f"ya{t}", tag=f"yacc{t}",
                                bufs=1) for t in range(2)]
                nc.vector.memset(ya[0][:, 0:2 * 130], 0.0)
                nc.vector.memset(ya[1][:, 0:2 * 130], 0.0)
                for tki in range(ntk):
                    t0 = b * seq + tki * 128
                    gti = t0 // 128
                    m = tki - qb * dtiles
                    # diagonal tile m covers only tq columns >= 128*m
                    c0 = 128 * m if m > 0 else 0
                    # both heads' scores side by side in one 2-bank tile
                    ps_s = psum.tile([128, hpc * tb], F32, name="ps_s",
                                     tag="s2", bufs=2)
                    p_t = work.tile([128, hpc * tb], BF16, name="p_t",
                                    tag="pt", bufs=6)
                    for h in range(hpc):
                        hs = slice(h * hd, (h + 1) * hd)
                        nc.tensor.matmul(ps_s[:, h * tb + c0:(h + 1) * tb],
                                         k_sb[hs, t0:t0 + 128],
                                         q_sb[hs, tq0 + c0:tq0 + tb],
                                         start=True, stop=True)
                    # one exp for both heads: AP [128, hpc, nq]
                    sv = ps_s[:].rearrange("p (h q) -> p h q", h=hpc)[:, :, c0:tb]
                    pv = p_t[:].rearrange("p (h q) -> p h q", h=hpc)[:, :, c0:tb]
                    nc.scalar.activation(pv, sv, AF.Exp, scale=scale)
                    if m >= 0:
                        # mask the [128,128] triangle at cols [c0, c0+128)
                        # (on DVE: Pool-engine work queues behind the 28us
                        # collectives and the monotonic Pool semaphore then
                        # stalls the PV Ldweights until the collective ends)
                        for h in range(hpc):
                            ap = p_t[:, h * tb + c0:h * tb + c0 + 128]
                            nc.vector.tensor_mul(ap, ap, tri[:])
                    for h in range(hpc):
                        vs = v_sb[:, gti * vw + h * (hd + 1):
                                  gti * vw + (h + 1) * (hd + 1)]
                        for qt in range(max(0, m), dtiles):
                            ti, g = divmod(qt, 2)
                            off = g * 130 + h * 65
                            nc.tensor.matmul(
                                ya[ti][:, off:off + 65],
                                p_t[:, h * tb + qt * 128:h * tb + (qt + 1) * 128],
                                vs, start=False,
                                stop=(tki == qb * dtiles + qt),
                                skip_group_check=True)
                # normalize: y_tok[:, gq, h*hd:...] = y/denom (token-major)
                for qt in range(dtiles):
                    ti, g = divmod(qt, 2)
                    gq = b * qpb + qb * dtiles + qt
                    for h in range(hpc):
                        off = g * 130 + h * 65
                        rcp = work.tile([128, 1], F32, name="rcp", tag="rcp",
                                        bufs=8)
                        nc.vector.reciprocal(rcp[:], ya[ti][:, off + 64:off + 65])
                        nc.vector.tensor_scalar_mul(
                            y_tok[:, gq * fl + h * hd:gq * fl + (h + 1) * hd],
                            ya[ti][:, off:off + 64], rcp[:])

            def a2a_issue(b):
                # shard j of batch b = qtiles b*qpb + [2j, 2j+2)
                for j in range(n_cores):
                    g0 = b * qpb + 2 * j
                    nc.gpsimd.dma_start(
                        a2a_in[b][j],
                        y_tok[:, g0 * fl:(g0 + 2) * fl])
                nc.gpsimd.collective_compute(
                    "AllToAll", mybir.AluOpType.bypass,
                    replica_groups=[list(range(n_cores))],
                    ins=[a2a_in[b][:]], outs=[a2a_out[b][:]],
                )

            def a2a_issue_half(b, hf):
                # shard j of half hf = qtile b*qpb + hf*8 + j
                for j in range(n_cores):
                    g0 = b * qpb + hf * (qpb // 2) + j
                    nc.gpsimd.dma_start(
                        a2ah_in[hf][j],
                        y_tok[:, g0 * fl:(g0 + 1) * fl])
                nc.gpsimd.collective_compute(
                    "AllToAll", mybir.AluOpType.bypass,
                    replica_groups=[list(range(n_cores))],
                    ins=[a2ah_in[hf][:]], outs=[a2ah_out[hf][:]],
                )

            def proj_half(pb, hf, not_before_ms):
                # output cols [pb*tsb + hf*tsh, pb*tsb + (hf+1)*tsh)
                with tc.tile_wait_until(not_before_ms):
                    y_loc = [work.tile([fl, tsh], BF16, name="y_loch",
                                       tag=f"ylh{i}", bufs=1) for i in range(n_cores)]
                    for i in range(n_cores):
                        nc.sync.dma_start_transpose(y_loc[i][:], a2ah_out[hf][i])
                    c0o = pb * tsb + hf * tsh
                    for dj in range(d // 128):
                        ps_o = psum.tile([128, tsh], F32, name="ps_oh", tag="mm512", bufs=2)
                        for i in range(n_cores):
                            nc.tensor.matmul(
                                ps_o[:], wp_sb[i][:, dj * 128:(dj + 1) * 128],
                                y_loc[i][:], start=(i == 0), stop=(i == n_cores - 1))
                        o_sb = work.tile([128, tsh], F32, name="o_sbh", tag="osb", bufs=3)
                        nc.scalar.copy(o_sb[:], ps_o[:])
                        nc.sync.dma_start(
                            out[dj * 128:(dj + 1) * 128, c0o:c0o + tsh], o_sb[:])

            def proj(pb, not_before_ms):
                with tc.tile_wait_until(not_before_ms):
                    y_loc = [work.tile([fl, tsb], BF16, name="y_loc",
                                       tag=f"yloc{i}", bufs=2) for i in range(n_cores)]
                    for i in range(n_cores):
                        for t in range(2):
                            nc.sync.dma_start_transpose(
                                y_loc[i][:, t * 128:(t + 1) * 128],
                                a2a_out[pb][i][:, t * 128:(t + 1) * 128])
                    for dj in range(d // 128):
                        ps_o = psum.tile([128, tsb], F32, name="ps_o", tag="mm512", bufs=2)
                        for i in range(n_cores):
                            nc.tensor.matmul(
                                ps_o[:], wp_sb[i][:, dj * 128:(dj + 1) * 128],
                                y_loc[i][:], start=(i == 0), stop=(i == n_cores - 1))
                        o_sb = work.tile([128, tsb], F32, name="o_sb", tag="osb", bufs=3)
                        nc.scalar.copy(o_sb[:], ps_o[:])
                        nc.sync.dma_start(
                            out[dj * 128:(dj + 1) * 128, pb * tsb:(pb + 1) * tsb],
                            o_sb[:])

            units = [(b, qb) for b in range(bsz) for qb in range(nqb)]

            # startup: first w pair (hi+lo), then unit-0 x, then rest of w
            def load_w8(j):
                rows = slice(2 * j * 128, (2 * j + 2) * 128)
                for t8, src in ((w_hi[j], wqh_d), (w_lo[j], wql_d)):
                    nc.sync.dma_start(
                        t8[:].rearrange("p (i c) -> p i c", i=2),
                        src[rows, :].rearrange("(i p) c -> p i c", p=128))
            load_w8(0)
            x_pre0 = load_x8(0)
            for j in range(1, npair):
                load_w8(j)
            qkv_block(*units[0], x_pre=x_pre0)
            if len(units) > 1:
                qkv_block(*units[1])
            for i in range(n_cores):
                nc.sync.dma_start(wp_sb[i][:], wproj[i * fl:(i + 1) * fl, :])
            last_b = bsz - 1
            # schedule hints: don't let the list scheduler hoist proj work
            # (which waits on its batch's collective) into the attention
            # stream before the collective can have finished.
            proj_ms = {0: 0.094, 1: 0.137, 2: 0.186}
            for L, (b, qb) in enumerate(units):
                attn_block(b, qb)
                if L + 2 < len(units):
                    qkv_block(*units[L + 2])
                if split_last and b == last_b and qb == nqb // 2 - 1:
                    a2a_issue_half(b, 0)
                if qb == nqb - 1:
                    if split_last and b == last_b:
                        a2a_issue_half(b, 1)
                    else:
                        a2a_issue(b)
                    if b >= 1:
                        proj(b - 1, proj_ms[b - 1])
            if split_last:
                proj_half(last_b, 0, 0.211)
                proj_half(last_b, 1, 0.2455)
            else:
                proj(bsz - 1, 0.2455)
    _split_multi_waits(nc)
    return nc


def _hi_lo(a, scale):
    """fp8 e4m3 residual pair of `scale*a`: a ~ (hi + lo)/scale."""
    f8 = mybir.dt.np(mybir.dt.float8e4)
    s = (scale * np.asarray(a, np.float32))
    hi = s.astype(f8)
    lo = (s - hi.astype(np.float32)).astype(f8)
    return np.ascontiguousarray(hi), np.ascontiguousarray(lo)


def shard_inputs(x, w_qkv, w_proj, n_cores=N_CORES, n_head=N_HEAD):
    bf16 = ml_dtypes.bfloat16
    d = x.shape[-1]
    T = x.shape[0] * x.shape[1]
    hpc = n_head // n_cores
    fl = hpc * HD
    xT = np.asarray(x, np.float32).reshape(T, d).T
    xh, xl = _hi_lo(xT, SX)
    wq = np.asarray(w_qkv, np.float32)
    wp = np.ascontiguousarray(np.asarray(w_proj, np.float32).T.astype(bf16))
    in_maps = []
    for c in range(n_cores):
        r0 = c * fl
        wqkv_c = np.concatenate(
            [wq[r0:r0 + fl], wq[d + r0:d + r0 + fl],
             wq[2 * d + r0:2 * d + r0 + fl]], axis=0).T
        wqh, wql = _hi_lo(wqkv_c, SW)
        in_maps.append({"xh": xh, "xl": xl, "wqh": wqh, "wql": wql,
                       "wproj": wp})
    return in_maps


def assemble_out(outs, n_cores=N_CORES, bsz=BSZ, seq=SEQ, d=D):
    """outs[c] is [d, bsz*tsb]; column block b holds tokens
    b*seq + [c*tsb, (c+1)*tsb) — except the last batch when nqb==4, which is
    exchanged in halves: tokens b*seq + h*seq/2 + [c*tsh, (c+1)*tsh)."""
    tsb = seq // n_cores
    tsh = tsb // 2
    split_last = (seq // min(512, seq)) == 4
    T = bsz * seq
    outT = np.empty((d, T), np.float32)
    for c in range(n_cores):
        for b in range(bsz):
            if split_last and b == bsz - 1:
                for h in range(2):
                    base = b * seq + h * (seq // 2)
                    outT[:, base + c * tsh:base + (c + 1) * tsh] = \
                        outs[c][:, b * tsb + h * tsh:b * tsb + (h + 1) * tsh]
            else:
                outT[:, b * seq + c * tsb:b * seq + (c + 1) * tsb] = \
                    outs[c][:, b * tsb:(b + 1) * tsb]
    return np.ascontiguousarray(outT.T).reshape(bsz, seq, d)


_NC_CACHE = {}


def kernel(x, w_qkv, w_proj):
    key = "full"
    if key not in _NC_CACHE:
        _NC_CACHE[key] = build_nc()
    nc = _NC_CACHE[key]
    in_maps = shard_inputs(x, w_qkv, w_proj)
    res = run_bass_kernel_spmd(nc, in_maps, list(range(N_CORES))).results
    return assemble_out([res[c]["out"] for c in range(N_CORES)]).astype(np.float32)


# revision 55
# speedup vs baseline: 1.2469x; 1.0236x over previous
"""Causal self-attention (dense transformer) on 8 TRN2 NeuronCores.

Sharding: heads+batch tensor-parallel. Each core c owns 2 heads (2c, 2c+1)
for all 4 batches:
  - QKV projection with the w_qkv row-slice for its heads (x is replicated,
    fed pre-transposed as xT [d, tokens] so d lands on SBUF partitions).
  - Causal attention for its 8 (batch, head) pairs, computed in
    "transposed scores" layout S_t[tk, tq]. PV matmuls are token-major
    (stationary P^T tile, moving [V | 1]) so the output free dim is 65 wide
    instead of 512: y accumulates per 128-token q-tile as [tq, hd+1] with the
    softmax denominator in the last column.
  - Normalization is a per-partition reciprocal (DVE) + broadcast multiply
    (Pool) into a token-major y_tok buffer.
  - Per-batch AllToAll exchanges y token-major; the receiver DMA-transposes
    each [tok, fl] chunk back to feature-major y_loc for the output
    projection (last batch exchanges in two halves so its collective
    overlaps).
Host side: shard/transpose/cast inputs, concat+transpose the output.

Compute dtype bf16 (PSUM accumulation fp32), storage fp32 in/out.
"""

import numpy as np
import ml_dtypes

import concourse.bass as bass
import concourse.mybir as mybir
import concourse.tile as tile
from concourse.bass_utils import run_bass_kernel_spmd

BF16 = mybir.dt.bfloat16
F32 = mybir.dt.float32
F8 = mybir.dt.float8e4
AF = mybir.ActivationFunctionType
DR = mybir.MatmulPerfMode.DoubleRow

# fp8 residual scales: x ~ (xh + xl)/SX, w ~ (wh + wl)/SW elementwise, with
# xh/xl/wh/wl e4m3 (max 240). qkv output scale = 1/(SX*SW).
SX = 16.0
SW = 256.0

# Full-size problem constants (hardcoded per harness contract)
N_CORES = 8
BSZ, SEQ, D, N_HEAD = 4, 2048, 1024, 16
HD = 64  # head dim


def _split_multi_waits(nc):
    """walrus on this build accepts at most ONE sync-wait command per
    instruction. Hoist extra waits onto standalone same-engine nops placed
    immediately before the instruction (queue order preserves semantics)."""
    edits = []
    for func in nc.m.functions:
        for bb in func.blocks:
            insts = bb.instructions
            for idx, ins in enumerate(insts):
                si = ins.sync_info
                if si is not None and len(si.on_wait) > 1:
                    edits.append((bb, idx, ins))
    for bb, idx, ins in reversed(edits):
        si = ins.sync_info
        extra, keep = list(si.on_wait[:-1]), [si.on_wait[-1]]
        ins.sync_info = mybir.SyncInfo(on_wait=keep, on_update=list(si.on_update))
        nops = []
        for w in extra:
            nop = nc.engines[ins.engine].nop().ins
            host = nc.cur_bb.bb.instructions
            assert host[-1] is nop
            host.pop()
            nop.sync_info = mybir.SyncInfo(on_wait=[w], on_update=[])
            nops.append(nop)
        live = bb.instructions
        for j, nop in enumerate(nops):
            live.insert(idx + j, nop)


def build_nc(n_cores=N_CORES, bsz=BSZ, seq=SEQ, d=D, n_head=N_HEAD):
    hd = HD
    hpc = n_head // n_cores          # heads per core
    fl = hpc * hd                    # local feature width (q/k/v per core)
    T = bsz * seq                    # total tokens
    kd = d // 128                    # contraction tiles over d
    tb = min(512, seq)               # tq block width (matmul free dim)
    nqb = seq // tb                  # q-blocks per batch
    dtiles = tb // 128               # 128-tiles per q-block
    nt = T // 128                    # total 128-token tiles
    scale = float(1.0 / np.sqrt(hd))

    tsb = seq // n_cores             # per-batch token chunk per core
    qpb = seq // 128                 # 128-token q-tiles per batch
    nc = bass.Bass(num_devices=n_cores)
    xh_d = nc.declare_dram_parameter("xh", [d, T], F8, isOutput=False)
    xl_d = nc.declare_dram_parameter("xl", [d, T], F8, isOutput=False)
    wqh_d = nc.declare_dram_parameter("wqh", [d, 3 * fl], F8, isOutput=False)
    wql_d = nc.declare_dram_parameter("wql", [d, 3 * fl], F8, isOutput=False)
    wproj = nc.declare_dram_parameter("wproj", [d, d], BF16, isOutput=False)
    out = nc.declare_dram_parameter("out", [d, bsz * tsb], F32, isOutput=True)
    split_last = (nqb == 4)
    # batches exchanged in halves (their collectives start earlier on the
    # serial collective resource, shrinking the end-of-kernel chain)
    split_batches = {bsz - 1} if split_last else set()
    tsh = tsb // 2
    a2a_in = [nc.dram_tensor(f"a2a_in{b}", [n_cores, 128, tsb], BF16)
              if b not in split_batches else None for b in range(bsz)]
    a2a_out = [nc.dram_tensor(f"a2a_out{b}", [n_cores, 128, tsb], BF16)
               if b not in split_batches else None for b in range(bsz)]
    a2ah_in = {(b, h): nc.dram_tensor(f"a2ah_in{b}_{h}", [n_cores, 128, tsh], BF16)
               for b in sorted(split_batches) for h in range(2)}
    a2ah_out = {(b, h): nc.dram_tensor(f"a2ah_out{b}_{h}", [n_cores, 128, tsh], BF16)
                for b in sorted(split_batches) for h in range(2)}

    with tile.TileContext(nc) as tc:
        with (
            tc.tile_pool(name="const", bufs=1) as const,
            tc.tile_pool(name="xin", bufs=2) as xin,
            tc.tile_pool(name="work", bufs=3) as work,
            tc.tile_pool(name="psum", bufs=1, space="PSUM") as psum,
        ):
            # ---- persistent SBUF ----
            # qkv weights as fp8 hi/lo, ktile-PAIR layout for DoubleRow:
            # w8[h][j] free dims (i, c): i = pair half (ktile 2j+i), c = col
            npair = kd // 2
            w_hi = [const.tile([128, 2 * 3 * fl], F8, name=f"w_hi{j}")
                    for j in range(npair)]
            w_lo = [const.tile([128, 2 * 3 * fl], F8, name=f"w_lo{j}")
                    for j in range(npair)]
            wp_sb = [const.tile([fl, d], BF16, name=f"wp_sb{i}") for i in range(n_cores)]

            q_sb = const.tile([fl, T], BF16, name="q_sb")
            k_sb = const.tile([fl, T], BF16, name="k_sb")
            # token-major y: [tok128, (global qtile, fl)]
            y_tok = const.tile([128, nt * fl], BF16, name="y_tok")
            # v token-major with a ones column per (tile, head): tile g, head h
            # occupies cols g*hpc*(hd+1) + h*(hd+1) + [0, hd+1); col hd is ones.
            vw = hpc * (hd + 1)
            v_sb = const.tile([128, nt * vw], BF16, name="v_sb")
            ones_ap = v_sb.rearrange("p (n h c) -> p n h c", h=hpc,
                                     c=hd + 1)[:, :, :, hd:hd + 1]
            nc.vector.memset(ones_ap, 1.0)

            # triangular mask [128,128]: keep S_t[tk_i, tq_j] iff i <= j
            tri = const.tile([128, 128], BF16, name="tri")
            nc.gpsimd.memset(tri[:], 1.0)
            nc.gpsimd.affine_select(
                out=tri[:], in_=tri[:],
                compare_op=mybir.AluOpType.is_ge, fill=0.0,
                base=0, channel_multiplier=-1, pattern=[[1, 128]],
            )

            # ---- unified (batch, q-block) stream with qkv prefetch ----
            def load_x8(ts0):
                x_hi = xin.tile([128, kd * tb], F8, name="x_hi", tag="xh", bufs=3)
                x_lo = xin.tile([128, kd * tb], F8, name="x_lo", tag="xl", bufs=3)
                # one DMA each: in [(j i p), T] -> out [128, j, i, tb]
                for t8, src in ((x_hi, xh_d), (x_lo, xl_d)):
                    nc.sync.dma_start(
                        t8[:].rearrange("p (j i t) -> p j i t", j=npair, i=2),
                        src.rearrange("(j i p) T -> p j i T", p=128,
                                      i=2)[:, :, :, ts0:ts0 + tb])
                return x_hi, x_lo

            def qkv_block(b, qb, x_pre=None):
                tbi = b * (seq // tb) + qb
                ts0 = tbi * tb
                if x_pre is None:
                    x_hi, x_lo = load_x8(ts0)
                else:
                    x_hi, x_lo = x_pre
                xh_r = x_hi[:].rearrange("p (j i t) -> p j i t", j=npair, i=2)
                xl_r = x_lo[:].rearrange("p (j i t) -> p j i t", j=npair, i=2)
                wh_r = [w_hi[j][:].rearrange("p (i c) -> p i c", i=2)
                        for j in range(npair)]
                wl_r = [w_lo[j][:].rearrange("p (i c) -> p i c", i=2)
                        for j in range(npair)]
                # q, k (feature-major): out [fl, tb], fp8 DoubleRow residual
                # (hi*hi + lo*hi + hi*lo; each instr contracts a ktile pair)
                for which, dst in ((0, q_sb), (1, k_sb)):
                    cs = slice(which * fl, (which + 1) * fl)
                    ps = psum.tile([fl, tb], F32, name=f"ps_qk{which}", tag="mm512", bufs=2)
                    n3 = 3 * npair
                    ii = 0
                    for j in range(npair):
                        for wop, xop in ((wh_r[j], xh_r), (wl_r[j], xh_r),
                                         (wh_r[j], xl_r)):
                            nc.tensor.matmul(
                                ps[:], wop[:, :, cs], xop[:, j],
                                start=(ii == 0), stop=(ii == n3 - 1),
                                perf_mode=DR)
                            ii += 1
                    nc.vector.tensor_scalar_mul(dst[:, ts0:ts0 + tb], ps[:],
                                                1.0 / (SX * SW))
                # v (token-major): out [128 tok, fl]
                vcs = slice(2 * fl, 3 * fl)
                for tt in range(dtiles):
                    gti = tbi * dtiles + tt
                    tts = slice(tt * 128, (tt + 1) * 128)
                    ps_v = psum.tile([128, fl], F32, name="ps_v", tag="mm512", bufs=2)
                    n3 = 3 * npair
                    ii = 0
                    for j in range(npair):
                        for xop, wop in ((xh_r, wh_r[j]), (xl_r, wh_r[j]),
                                         (xh_r, wl_r[j])):
                            nc.tensor.matmul(
                                ps_v[:], xop[:, j, :, tts], wop[:, :, vcs],
                                start=(ii == 0), stop=(ii == n3 - 1),
                                perf_mode=DR)
                            ii += 1
                    nc.vector.tensor_scalar_mul(
                        v_sb.rearrange("p (n h c) -> p n h c", h=hpc,
                                       c=hd + 1)[:, gti, :, 0:hd],
                        ps_v[:].rearrange("p (h c) -> p h c", c=hd),
                        1.0 / (SX * SW))

            # last-emitted attention instructions per engine; late proj work
            # gets explicit deps on these so the scheduler cannot interleave
            # it ahead of attention in the Act/PE queues (which deadlock-ish
            # head-of-line-blocks both queues until a collective resolves it)
            attn_last = {"exp": None, "pv": None}

            def attn_block(b, qb):
                tq0 = b * seq + qb * tb
                ntk = (qb + 1) * dtiles
                # y accumulators: 2 one-bank tiles, 4 groups (2 qt x 2 heads)
                # each; group (qt, h) at cols (qt%2)*130 + h*65, width 65
                # (64 y + 1 denom). All matmuls accumulate (start=False) onto
                # an explicit memset so group order is irrelevant.
                ya = [psum.tile([128, 512], F32, name=f"ya{t}", tag=f"yacc{t}",
                                bufs=1) for t in range(2)]
                nc.vector.memset(ya[0][:, 0:2 * 130], 0.0)
                nc.vector.memset(ya[1][:, 0:2 * 130], 0.0)
                for tki in range(ntk):
                    t0 = b * seq + tki * 128
                    gti = t0 // 128
                    m = tki - qb * dtiles
                    # diagonal tile m covers only tq columns >= 128*m
                    c0 = 128 * m if m > 0 else 0
                    # both heads' scores side by side in one 2-bank tile
                    ps_s = psum.tile([128, hpc * tb], F32, name="ps_s",
                                     tag="s2", bufs=2)
                    p_t = work.tile([128, hpc * tb], BF16, name="p_t",
                                    tag="pt", bufs=6)
                    for h in range(hpc):
                        hs = slice(h * hd, (h + 1) * hd)
                        nc.tensor.matmul(ps_s[:, h * tb + c0:(h + 1) * tb],
                                         k_sb[hs, t0:t0 + 128],
                                         q_sb[hs, tq0 + c0:tq0 + tb],
                                         start=True, stop=True)
                    # one exp for both heads: AP [128, hpc, nq]
                    sv = ps_s[:].rearrange("p (h q) -> p h q", h=hpc)[:, :, c0:tb]
                    pv = p_t[:].rearrange("p (h q) -> p h q", h=hpc)[:, :, c0:tb]
                    attn_last["exp"] = nc.scalar.activation(
                        pv, sv, AF.Exp, scale=scale)
                    if m >= 0:
                        # mask the [128,128] triangle at cols [c0, c0+128)
                        # (on DVE: Pool-engine work queues behind the 28us
                        # collectives and the monotonic Pool semaphore then
                        # stalls the PV Ldweights)
                        for h in range(hpc):
                            ap = p_t[:, h * tb + c0:h * tb + c0 + 128]
                            nc.vector.tensor_mul(ap, ap, tri[:])
                    for h in range(hpc):
                        vs = v_sb[:, gti * vw + h * (hd + 1):
                                  gti * vw + (h + 1) * (hd + 1)]
                        for qt in range(max(0, m), dtiles):
                            ti, g = divmod(qt, 2)
                            off = g * 130 + h * 65
                            attn_last["pv"] = nc.tensor.matmul(
                                ya[ti][:, off:off + 65],
                                p_t[:, h * tb + qt * 128:h * tb + (qt + 1) * 128],
                                vs, start=False,
                                stop=(tki == qb * dtiles + qt),
                                skip_group_check=True)
                # normalize: y_tok[:, gq, h*hd:...] = y/denom (token-major)
                for qt in range(dtiles):
                    ti, g = divmod(qt, 2)
                    gq = b * qpb + qb * dtiles + qt
                    for h in range(hpc):
                        off = g * 130 + h * 65
                        rcp = work.tile([128, 1], F32, name="rcp", tag="rcp",
                                        bufs=8)
                        nc.vector.reciprocal(rcp[:], ya[ti][:, off + 64:off + 65])
                        nc.vector.tensor_scalar_mul(
                            y_tok[:, gq * fl + h * hd:gq * fl + (h + 1) * hd],
                            ya[ti][:, off:off + 64], rcp[:])

            def a2a_issue(b):
                # shard j of batch b = qtiles b*qpb + [2j, 2j+2)
                for j in range(n_cores):
                    g0 = b * qpb + 2 * j
                    nc.gpsimd.dma_start(
                        a2a_in[b][j],
                        y_tok[:, g0 * fl:(g0 + 2) * fl])
                nc.gpsimd.collective_compute(
                    "AllToAll", mybir.AluOpType.bypass,
                    replica_groups=[list(range(n_cores))],
                    ins=[a2a_in[b][:]], outs=[a2a_out[b][:]],
                )

            def a2a_issue_half(b, hf):
                # shard j of half hf = qtile b*qpb + hf*8 + j
                for j in range(n_cores):
                    g0 = b * qpb + hf * (qpb // 2) + j
                    nc.gpsimd.dma_start(
                        a2ah_in[(b, hf)][j],
                        y_tok[:, g0 * fl:(g0 + 1) * fl])
                nc.gpsimd.collective_compute(
                    "AllToAll", mybir.AluOpType.bypass,
                    replica_groups=[list(range(n_cores))],
                    ins=[a2ah_in[(b, hf)][:]], outs=[a2ah_out[(b, hf)][:]],
                )

            from contextlib import nullcontext
            from concourse.tile_rust import add_dep_helper

            def _after_attn(bi, kind):
                # pin a late-proj instruction behind the final attention
                # work on its engine queue
                ref = attn_last[kind]
                if ref is not None:
                    add_dep_helper(bi.ins, ref.ins, sync=True,
                                   reason="proj after attention")

            def proj_gate(not_before_ms):
                # floor proj work at a scheduler timestamp: keeps it out of
                # the attention stream until its collective can be done, but
                # not so late that the shared mm512-psum FIFO stalls later
                # qkv blocks
                if not_before_ms > 0:
                    return tc.tile_wait_until(not_before_ms)
                return nullcontext()

            def proj_half(pb, hf, not_before_ms, after_attn=False):
                # output cols [pb*tsb + hf*tsh, pb*tsb + (hf+1)*tsh)
                with proj_gate(not_before_ms):
                    y_loc = [work.tile([fl, tsh], BF16, name="y_loch",
                                       tag=f"ylh{i}", bufs=2) for i in range(n_cores)]
                    for i in range(n_cores):
                        nc.sync.dma_start_transpose(y_loc[i][:],
                                                    a2ah_out[(pb, hf)][i])
                    c0o = pb * tsb + hf * tsh
                    for dj in range(d // 128):
                        ps_o = psum.tile([128, tsh], F32, name="ps_oh", tag="mm512", bufs=2)
                        for i in range(n_cores):
                            mm = nc.tensor.matmul(
                                ps_o[:], wp_sb[i][:, dj * 128:(dj + 1) * 128],
                                y_loc[i][:], start=(i == 0), stop=(i == n_cores - 1))
                            if after_attn:
                                _after_attn(mm, "pv")
                        o_sb = work.tile([128, tsh], F32, name="o_sbh",
                                         tag="osb", bufs=3)
                        cp = nc.scalar.copy(o_sb[:], ps_o[:])
                        if after_attn:
                            _after_attn(cp, "exp")
                        nc.sync.dma_start(
                            out[dj * 128:(dj + 1) * 128, c0o:c0o + tsh],
                            o_sb[:])

            def proj(pb, not_before_ms, after_attn=False):
                with proj_gate(not_before_ms):
                    y_loc = [work.tile([fl, tsb], BF16, name="y_loc",
                                       tag=f"yloc{i}", bufs=2) for i in range(n_cores)]
                    for i in range(n_cores):
                        for t in range(2):
                            nc.sync.dma_start_transpose(
                                y_loc[i][:, t * 128:(t + 1) * 128],
                                a2a_out[pb][i][:, t * 128:(t + 1) * 128])
                    for dj in range(d // 128):
                        ps_o = psum.tile([128, tsb], F32, name="ps_o", tag="mm512", bufs=2)
                        for i in range(n_cores):
                            mm = nc.tensor.matmul(
                                ps_o[:], wp_sb[i][:, dj * 128:(dj + 1) * 128],
                                y_loc[i][:], start=(i == 0), stop=(i == n_cores - 1))
                            if after_attn:
                                _after_attn(mm, "pv")
                        o_sb = work.tile([128, tsb], F32, name="o_sb",
                                         tag="osb", bufs=3)
                        cp = nc.scalar.copy(o_sb[:], ps_o[:])
                        if after_attn:
                            _after_attn(cp, "exp")
                        nc.sync.dma_start(
                            out[dj * 128:(dj + 1) * 128, pb * tsb:(pb + 1) * tsb],
                            o_sb[:])

            units = [(b, qb) for b in range(bsz) for qb in range(nqb)]

            # startup: first w pair (hi+lo), then unit-0 x, then rest of w
            def load_w8(j):
                rows = slice(2 * j * 128, (2 * j + 2) * 128)
                for t8, src in ((w_hi[j], wqh_d), (w_lo[j], wql_d)):
                    nc.sync.dma_start(
                        t8[:].rearrange("p (i c) -> p i c", i=2),
                        src[rows, :].rearrange("(i p) c -> p i c", p=128))
            load_w8(0)
            x_pre0 = load_x8(0)
            for j in range(1, npair):
                load_w8(j)
            qkv_block(*units[0], x_pre=x_pre0)
            if len(units) > 1:
                qkv_block(*units[1])
            for i in range(n_cores):
                nc.sync.dma_start(wp_sb[i][:], wproj[i * fl:(i + 1) * fl, :])
            last_b = bsz - 1
            # schedule hints: don't let the list scheduler hoist proj work
            # (which waits on its batch's collective) into the attention
            # stream before the collective can have finished.
            import os as _os
            _f = [float(v) for v in _os.environ.get(
                "PROJ_MS", "0.094,0.137,0.186,0,0.211,0.229").split(",")]
            for L, (b, qb) in enumerate(units):
                attn_block(b, qb)
                if L + 2 < len(units):
                    qkv_block(*units[L + 2])
                if b in split_batches and qb == nqb // 2 - 1:
                    a2a_issue_half(b, 0)
                if qb == nqb - 1:
                    if b in split_batches:
                        a2a_issue_half(b, 1)
                    else:
                        a2a_issue(b)
                    if b >= 1 and b - 1 not in split_batches:
                        proj(b - 1, _f[b - 1], after_attn=(b == last_b))
            if split_last:
                proj_half(last_b, 0, _f[4], after_attn=True)
                proj_half(last_b, 1, _f[5], after_attn=True)
            else:
                proj(bsz - 1, _f[5], after_attn=True)
    _split_multi_waits(nc)
    return nc


def _hi_lo(a, scale):
    """fp8 e4m3 residual pair of `scale*a`: a ~ (hi + lo)/scale."""
    f8 = mybir.dt.np(mybir.dt.float8e4)
    s = (scale * np.asarray(a, np.float32))
    hi = s.astype(f8)
    lo = (s - hi.astype(np.float32)).astype(f8)
    return np.ascontiguousarray(hi), np.ascontiguousarray(lo)


def shard_inputs(x, w_qkv, w_proj, n_cores=N_CORES, n_head=N_HEAD):
    bf16 = ml_dtypes.bfloat16
    d = x.shape[-1]
    T = x.shape[0] * x.shape[1]
    hpc = n_head // n_cores
    fl = hpc * HD
    xT = np.asarray(x, np.float32).reshape(T, d).T
    xh, xl = _hi_lo(xT, SX)
    wq = np.asarray(w_qkv, np.float32)
    wp = np.ascontiguousarray(np.asarray(w_proj, np.float32).T.astype(bf16))
    in_maps = []
    for c in range(n_cores):
        r0 = c * fl
        wqkv_c = np.concatenate(
            [wq[r0:r0 + fl], wq[d + r0:d + r0 + fl],
             wq[2 * d + r0:2 * d + r0 + fl]], axis=0).T
        wqh, wql = _hi_lo(wqkv_c, SW)
        in_maps.append({"xh": xh, "xl": xl, "wqh": wqh, "wql": wql,
                       "wproj": wp})
    return in_maps


def assemble_out(outs, n_cores=N_CORES, bsz=BSZ, seq=SEQ, d=D):
    """outs[c] is [d, bsz*tsb]; column block b holds tokens
    b*seq + [c*tsb, (c+1)*tsb) — except half-exchanged batches (the last two
    when nqb==4): tokens b*seq + h*seq/2 + [c*tsh, (c+1)*tsh)."""
    tsb = seq // n_cores
    tsh = tsb // 2
    split_last = (seq // min(512, seq)) == 4
    split_batches = {bsz - 1} if split_last else set()
    T = bsz * seq
    outT = np.empty((d, T), np.float32)
    for c in range(n_cores):
        for b in range(bsz):
            if b in split_batches:
                for h in range(2):
                    base = b * seq + h * (seq // 2)
                    outT[:, base + c * tsh:base + (c + 1) * tsh] = \
                        outs[c][:, b * tsb + h * tsh:b * tsb + (h + 1) * tsh]
            else:
                outT[:, b * seq + c * tsb:b * seq + (c + 1) * tsb] = \
                    outs[c][:, b * tsb:(b + 1) * tsb]
    return np.ascontiguousarray(outT.T).reshape(bsz, seq, d)


_NC_CACHE = {}


def kernel(x, w_qkv, w_proj):
    key = "full"
    if key not in _NC_CACHE:
        _NC_CACHE[key] = build_nc()
    nc = _NC_CACHE[key]
    in_maps = shard_inputs(x, w_qkv, w_proj)
    res = run_bass_kernel_spmd(nc, in_maps, list(range(N_CORES))).results
    return assemble_out([res[c]["out"] for c in range(N_CORES)]).astype(np.float32)


# revision 58
# speedup vs baseline: 1.2651x; 1.0146x over previous
"""Causal self-attention (dense transformer) on 8 TRN2 NeuronCores.

Sharding: heads+batch tensor-parallel. Each core c owns 2 heads (2c, 2c+1)
for all 4 batches:
  - QKV projection with the w_qkv row-slice for its heads (x is replicated,
    fed pre-transposed as xT [d, tokens] so d lands on SBUF partitions).
  - Causal attention for its 8 (batch, head) pairs, computed in
    "transposed scores" layout S_t[tk, tq]. PV matmuls are token-major
    (stationary P^T tile, moving [V | 1]) so the output free dim is 65 wide
    instead of 512: y accumulates per 128-token q-tile as [tq, hd+1] with the
    softmax denominator in the last column.
  - Normalization is a per-partition reciprocal (DVE) + broadcast multiply
    (Pool) into a token-major y_tok buffer.
  - Per-batch AllToAll exchanges y token-major; the receiver DMA-transposes
    each [tok, fl] chunk back to feature-major y_loc for the output
    projection (last batch exchanges in two halves so its collective
    overlaps).
Host side: shard/transpose/cast inputs, concat+transpose the output.

Compute dtype bf16 (PSUM accumulation fp32), storage fp32 in/out.
"""

import numpy as np
import ml_dtypes

import concourse.bass as bass
import concourse.mybir as mybir
import concourse.tile as tile
from concourse.bass_utils import run_bass_kernel_spmd

BF16 = mybir.dt.bfloat16
F32 = mybir.dt.float32
F8 = mybir.dt.float8e4
AF = mybir.ActivationFunctionType
DR = mybir.MatmulPerfMode.DoubleRow

# fp8 residual scales: x ~ (xh + xl)/SX, w ~ (wh + wl)/SW elementwise, with
# xh/xl/wh/wl e4m3 (max 240). qkv output scale = 1/(SX*SW).
SX = 16.0
SW = 256.0

# Full-size problem constants (hardcoded per harness contract)
N_CORES = 8
BSZ, SEQ, D, N_HEAD = 4, 2048, 1024, 16
HD = 64  # head dim


def _split_multi_waits(nc):
    """walrus on this build accepts at most ONE sync-wait command per
    instruction. Hoist extra waits onto standalone same-engine nops placed
    immediately before the instruction (queue order preserves semantics)."""
    edits = []
    for func in nc.m.functions:
        for bb in func.blocks:
            insts = bb.instructions
            for idx, ins in enumerate(insts):
                si = ins.sync_info
                if si is not None and len(si.on_wait) > 1:
                    edits.append((bb, idx, ins))
    for bb, idx, ins in reversed(edits):
        si = ins.sync_info
        extra, keep = list(si.on_wait[:-1]), [si.on_wait[-1]]
        ins.sync_info = mybir.SyncInfo(on_wait=keep, on_update=list(si.on_update))
        nops = []
        for w in extra:
            nop = nc.engines[ins.engine].nop().ins
            host = nc.cur_bb.bb.instructions
            assert host[-1] is nop
            host.pop()
            nop.sync_info = mybir.SyncInfo(on_wait=[w], on_update=[])
            nops.append(nop)
        live = bb.instructions
        for j, nop in enumerate(nops):
            live.insert(idx + j, nop)


def build_nc(n_cores=N_CORES, bsz=BSZ, seq=SEQ, d=D, n_head=N_HEAD):
    hd = HD
    hpc = n_head // n_cores          # heads per core
    fl = hpc * hd                    # local feature width (q/k/v per core)
    T = bsz * seq                    # total tokens
    kd = d // 128                    # contraction tiles over d
    tb = min(512, seq)               # tq block width (matmul free dim)
    nqb = seq // tb                  # q-blocks per batch
    dtiles = tb // 128               # 128-tiles per q-block
    nt = T // 128                    # total 128-token tiles
    scale = float(1.0 / np.sqrt(hd))

    tsb = seq // n_cores             # per-batch token chunk per core
    qpb = seq // 128                 # 128-token q-tiles per batch
    nc = bass.Bass(num_devices=n_cores)
    xh_d = nc.declare_dram_parameter("xh", [d, T], F8, isOutput=False)
    xl_d = nc.declare_dram_parameter("xl", [d, T], F8, isOutput=False)
    wqh_d = nc.declare_dram_parameter("wqh", [d, 3 * fl], F8, isOutput=False)
    wql_d = nc.declare_dram_parameter("wql", [d, 3 * fl], F8, isOutput=False)
    wproj = nc.declare_dram_parameter("wproj", [d, d], BF16, isOutput=False)
    out = nc.declare_dram_parameter("out", [d, bsz * tsb], F32, isOutput=True)
    split_last = (nqb == 4)
    # batches exchanged in halves (their collectives start earlier on the
    # serial collective resource, shrinking the end-of-kernel chain)
    split_batches = {bsz - 1} if split_last else set()
    tsh = tsb // 2
    a2a_in = [nc.dram_tensor(f"a2a_in{b}", [n_cores, 128, tsb], BF16)
              if b not in split_batches else None for b in range(bsz)]
    a2a_out = [nc.dram_tensor(f"a2a_out{b}", [n_cores, 128, tsb], BF16)
               if b not in split_batches else None for b in range(bsz)]
    a2ah_in = {(b, h): nc.dram_tensor(f"a2ah_in{b}_{h}", [n_cores, 128, tsh], BF16)
               for b in sorted(split_batches) for h in range(2)}
    a2ah_out = {(b, h): nc.dram_tensor(f"a2ah_out{b}_{h}", [n_cores, 128, tsh], BF16)
                for b in sorted(split_batches) for h in range(2)}

    with tile.TileContext(nc) as tc:
        with (
            tc.tile_pool(name="const", bufs=1) as const,
            tc.tile_pool(name="xin", bufs=2) as xin,
            tc.tile_pool(name="work", bufs=3) as work,
            tc.tile_pool(name="psum", bufs=1, space="PSUM") as psum,
        ):
            # ---- persistent SBUF ----
            # qkv weights as fp8 hi/lo, ktile-PAIR layout for DoubleRow:
            # w8[h][j] free dims (i, c): i = pair half (ktile 2j+i), c = col
            npair = kd // 2
            w_hi = [const.tile([128, 2 * 3 * fl], F8, name=f"w_hi{j}")
                    for j in range(npair)]
            w_lo = [const.tile([128, 2 * 3 * fl], F8, name=f"w_lo{j}")
                    for j in range(npair)]
            wp_sb = [const.tile([fl, d], BF16, name=f"wp_sb{i}") for i in range(n_cores)]

            q_sb = const.tile([fl, T], BF16, name="q_sb")
            k_sb = const.tile([fl, T], BF16, name="k_sb")
            # token-major y: [tok128, (global qtile, fl)]
            y_tok = const.tile([128, nt * fl], BF16, name="y_tok")
            # v token-major with a ones column per (tile, head): tile g, head h
            # occupies cols g*hpc*(hd+1) + h*(hd+1) + [0, hd+1); col hd is ones.
            vw = hpc * (hd + 1)
            v_sb = const.tile([128, nt * vw], BF16, name="v_sb")
            ones_ap = v_sb.rearrange("p (n h c) -> p n h c", h=hpc,
                                     c=hd + 1)[:, :, :, hd:hd + 1]
            nc.vector.memset(ones_ap, 1.0)

            # triangular mask [128,128]: keep S_t[tk_i, tq_j] iff i <= j
            tri = const.tile([128, 128], BF16, name="tri")
            nc.gpsimd.memset(tri[:], 1.0)
            nc.gpsimd.affine_select(
                out=tri[:], in_=tri[:],
                compare_op=mybir.AluOpType.is_ge, fill=0.0,
                base=0, channel_multiplier=-1, pattern=[[1, 128]],
            )

            # ---- unified (batch, q-block) stream with qkv prefetch ----
            def load_x8(ts0):
                x_hi = xin.tile([128, kd * tb], F8, name="x_hi", tag="xh", bufs=3)
                x_lo = xin.tile([128, kd * tb], F8, name="x_lo", tag="xl", bufs=3)
                # one DMA each: in [(j i p), T] -> out [128, j, i, tb]
                for t8, src in ((x_hi, xh_d), (x_lo, xl_d)):
                    nc.sync.dma_start(
                        t8[:].rearrange("p (j i t) -> p j i t", j=npair, i=2),
                        src.rearrange("(j i p) T -> p j i T", p=128,
                                      i=2)[:, :, :, ts0:ts0 + tb])
                return x_hi, x_lo

            def qkv_block(b, qb, x_pre=None):
                tbi = b * (seq // tb) + qb
                ts0 = tbi * tb
                if x_pre is None:
                    x_hi, x_lo = load_x8(ts0)
                else:
                    x_hi, x_lo = x_pre
                xh_r = x_hi[:].rearrange("p (j i t) -> p j i t", j=npair, i=2)
                xl_r = x_lo[:].rearrange("p (j i t) -> p j i t", j=npair, i=2)
                wh_r = [w_hi[j][:].rearrange("p (i c) -> p i c", i=2)
                        for j in range(npair)]
                wl_r = [w_lo[j][:].rearrange("p (i c) -> p i c", i=2)
                        for j in range(npair)]
                # q, k (feature-major): out [fl, tb], fp8 DoubleRow residual
                # (hi*hi + lo*hi + hi*lo; each instr contracts a ktile pair)
                for which, dst in ((0, q_sb), (1, k_sb)):
                    cs = slice(which * fl, (which + 1) * fl)
                    ps = psum.tile([fl, tb], F32, name=f"ps_qk{which}", tag="mm512", bufs=2)
                    n3 = 3 * npair
                    ii = 0
                    for j in range(npair):
                        for wop, xop in ((wh_r[j], xh_r), (wl_r[j], xh_r),
                                         (wh_r[j], xl_r)):
                            nc.tensor.matmul(
                                ps[:], wop[:, :, cs], xop[:, j],
                                start=(ii == 0), stop=(ii == n3 - 1),
                                perf_mode=DR)
                            ii += 1
                    nc.vector.tensor_scalar_mul(dst[:, ts0:ts0 + tb], ps[:],
                                                1.0 / (SX * SW))
                # v (token-major): out [128 tok, fl]
                vcs = slice(2 * fl, 3 * fl)
                for tt in range(dtiles):
                    gti = tbi * dtiles + tt
                    tts = slice(tt * 128, (tt + 1) * 128)
                    ps_v = psum.tile([128, fl], F32, name="ps_v", tag="mm512", bufs=2)
                    n3 = 3 * npair
                    ii = 0
                    for j in range(npair):
                        for xop, wop in ((xh_r, wh_r[j]), (xl_r, wh_r[j]),
                                         (xh_r, wl_r[j])):
                            nc.tensor.matmul(
                                ps_v[:], xop[:, j, :, tts], wop[:, :, vcs],
                                start=(ii == 0), stop=(ii == n3 - 1),
                                perf_mode=DR)
                            ii += 1
                    nc.vector.tensor_scalar_mul(
                        v_sb.rearrange("p (n h c) -> p n h c", h=hpc,
                                       c=hd + 1)[:, gti, :, 0:hd],
                        ps_v[:].rearrange("p (h c) -> p h c", c=hd),
                        1.0 / (SX * SW))

            # last-emitted attention instructions per engine; late proj work
            # gets explicit deps on these so the scheduler cannot interleave
            # it ahead of attention in the Act/PE queues (which deadlock-ish
            # head-of-line-blocks both queues until a collective resolves it)
            attn_last = {"exp": None, "pv": None}

            def attn_block(b, qb):
                tq0 = b * seq + qb * tb
                ntk = (qb + 1) * dtiles
                # y accumulators: 2 one-bank tiles, 4 groups (2 qt x 2 heads)
                # each; group (qt, h) at cols (qt%2)*130 + h*65, width 65
                # (64 y + 1 denom). All matmuls accumulate (start=False) onto
                # an explicit memset so group order is irrelevant.
                ya = [psum.tile([128, 512], F32, name=f"ya{t}", tag=f"yacc{t}",
                                bufs=1) for t in range(2)]
                nc.vector.memset(ya[0][:, 0:2 * 130], 0.0)
                nc.vector.memset(ya[1][:, 0:2 * 130], 0.0)
                for tki in range(ntk):
                    t0 = b * seq + tki * 128
                    gti = t0 // 128
                    m = tki - qb * dtiles
                    # diagonal tile m covers only tq columns >= 128*m
                    c0 = 128 * m if m > 0 else 0
                    # both heads' scores side by side in one 2-bank tile
                    ps_s = psum.tile([128, hpc * tb], F32, name="ps_s",
                                     tag="s2", bufs=2)
                    p_t = work.tile([128, hpc * tb], BF16, name="p_t",
                                    tag="pt", bufs=6)
                    for h in range(hpc):
                        hs = slice(h * hd, (h + 1) * hd)
                        nc.tensor.matmul(ps_s[:, h * tb + c0:(h + 1) * tb],
                                         k_sb[hs, t0:t0 + 128],
                                         q_sb[hs, tq0 + c0:tq0 + tb],
                                         start=True, stop=True)
                    # one exp for both heads: AP [128, hpc, nq]
                    sv = ps_s[:].rearrange("p (h q) -> p h q", h=hpc)[:, :, c0:tb]
                    pv = p_t[:].rearrange("p (h q) -> p h q", h=hpc)[:, :, c0:tb]
                    attn_last["exp"] = nc.scalar.activation(
                        pv, sv, AF.Exp, scale=scale)
                    if m >= 0:
                        # mask the [128,128] triangle at cols [c0, c0+128)
                        # (on DVE: Pool-engine work queues behind the 28us
                        # collectives and the monotonic Pool semaphore then
                        # stalls the PV Ldweights)
                        for h in range(hpc):
                            ap = p_t[:, h * tb + c0:h * tb + c0 + 128]
                            nc.vector.tensor_mul(ap, ap, tri[:])
                    for h in range(hpc):
                        vs = v_sb[:, gti * vw + h * (hd + 1):
                                  gti * vw + (h + 1) * (hd + 1)]
                        for qt in range(max(0, m), dtiles):
                            ti, g = divmod(qt, 2)
                            off = g * 130 + h * 65
                            attn_last["pv"] = nc.tensor.matmul(
                                ya[ti][:, off:off + 65],
                                p_t[:, h * tb + qt * 128:h * tb + (qt + 1) * 128],
                                vs, start=False,
                                stop=(tki == qb * dtiles + qt),
                                skip_group_check=True)
                # normalize: y_tok[:, gq, h*hd:...] = y/denom (token-major)
                for qt in range(dtiles):
                    ti, g = divmod(qt, 2)
                    gq = b * qpb + qb * dtiles + qt
                    for h in range(hpc):
                        off = g * 130 + h * 65
                        rcp = work.tile([128, 1], F32, name="rcp", tag="rcp",
                                        bufs=8)
                        nc.vector.reciprocal(rcp[:], ya[ti][:, off + 64:off + 65])
                        nc.vector.tensor_scalar_mul(
                            y_tok[:, gq * fl + h * hd:gq * fl + (h + 1) * hd],
                            ya[ti][:, off:off + 64], rcp[:])

            def a2a_issue(b):
                # shard j of batch b = qtiles b*qpb + [2j, 2j+2)
                for j in range(n_cores):
                    g0 = b * qpb + 2 * j
                    nc.gpsimd.dma_start(
                        a2a_in[b][j],
                        y_tok[:, g0 * fl:(g0 + 2) * fl])
                nc.gpsimd.collective_compute(
                    "AllToAll", mybir.AluOpType.bypass,
                    replica_groups=[list(range(n_cores))],
                    ins=[a2a_in[b][:]], outs=[a2a_out[b][:]],
                )

            def a2a_issue_half(b, hf):
                # shard j of half hf = qtile b*qpb + hf*8 + j
                for j in range(n_cores):
                    g0 = b * qpb + hf * (qpb // 2) + j
                    nc.gpsimd.dma_start(
                        a2ah_in[(b, hf)][j],
                        y_tok[:, g0 * fl:(g0 + 1) * fl])
                nc.gpsimd.collective_compute(
                    "AllToAll", mybir.AluOpType.bypass,
                    replica_groups=[list(range(n_cores))],
                    ins=[a2ah_in[(b, hf)][:]], outs=[a2ah_out[(b, hf)][:]],
                )

            from contextlib import nullcontext
            from concourse.tile_rust import add_dep_helper

            def _after_attn(bi, kind):
                # pin a late-proj instruction behind the final attention
                # work on its engine queue
                ref = attn_last[kind]
                if ref is not None:
                    add_dep_helper(bi.ins, ref.ins, sync=True,
                                   reason="proj after attention")

            def proj_gate(not_before_ms):
                # floor proj work at a scheduler timestamp: keeps it out of
                # the attention stream until its collective can be done, but
                # not so late that the shared mm512-psum FIFO stalls later
                # qkv blocks
                if not_before_ms > 0:
                    return tc.tile_wait_until(not_before_ms)
                return nullcontext()

            def proj_half(pb, hf, not_before_ms, after_attn=False):
                # output cols [pb*tsb + hf*tsh, pb*tsb + (hf+1)*tsh)
                with proj_gate(not_before_ms):
                    y_loc = [work.tile([fl, tsh], BF16, name="y_loch",
                                       tag=f"ylh{i}", bufs=2) for i in range(n_cores)]
                    for i in range(n_cores):
                        nc.sync.dma_start_transpose(y_loc[i][:],
                                                    a2ah_out[(pb, hf)][i])
                    c0o = pb * tsb + hf * tsh
                    for dj in range(d // 128):
                        ps_o = psum.tile([128, tsh], F32, name="ps_oh", tag="mm512", bufs=2)
                        for i in range(n_cores):
                            mm = nc.tensor.matmul(
                                ps_o[:], wp_sb[i][:, dj * 128:(dj + 1) * 128],
                                y_loc[i][:], start=(i == 0), stop=(i == n_cores - 1))
                            if after_attn:
                                _after_attn(mm, "pv")
                        o_sb = work.tile([128, tsh], F32, name="o_sbh",
                                         tag="osb", bufs=3)
                        cp = nc.scalar.copy(o_sb[:], ps_o[:])
                        if after_attn:
                            _after_attn(cp, "exp")
                        nc.sync.dma_start(
                            out[dj * 128:(dj + 1) * 128, c0o:c0o + tsh],
                            o_sb[:])

            def proj(pb, not_before_ms, after_attn=False):
                with proj_gate(not_before_ms):
                    y_loc = [work.tile([fl, tsb], BF16, name="y_loc",
                                       tag=f"yloc{i}", bufs=2) for i in range(n_cores)]
                    for i in range(n_cores):
                        for t in range(2):
                            nc.sync.dma_start_transpose(
                                y_loc[i][:, t * 128:(t + 1) * 128],
                                a2a_out[pb][i][:, t * 128:(t + 1) * 128])
                    for dj in range(d // 128):
                        ps_o = psum.tile([128, tsb], F32, name="ps_o", tag="mm512", bufs=2)
                        for i in range(n_cores):
                            mm = nc.tensor.matmul(
                                ps_o[:], wp_sb[i][:, dj * 128:(dj + 1) * 128],
                                y_loc[i][:], start=(i == 0), stop=(i == n_cores - 1))
                            if after_attn:
                                _after_attn(mm, "pv")
                        o_sb = work.tile([128, tsb], F32, name="o_sb",
                                         tag="osb", bufs=3)
                        cp = nc.scalar.copy(o_sb[:], ps_o[:])
                        if after_attn:
                            _after_attn(cp, "exp")
                        nc.sync.dma_start(
                            out[dj * 128:(dj + 1) * 128, pb * tsb:(pb + 1) * tsb],
                            o_sb[:])

            units = [(b, qb) for b in range(bsz) for qb in range(nqb)]

            # startup: first w pair (hi+lo), then unit-0 x, then rest of w
            def load_w8(j):
                rows = slice(2 * j * 128, (2 * j + 2) * 128)
                for t8, src in ((w_hi[j], wqh_d), (w_lo[j], wql_d)):
                    nc.sync.dma_start(
                        t8[:].rearrange("p (i c) -> p i c", i=2),
                        src[rows, :].rearrange("(i p) c -> p i c", p=128))
            load_w8(0)
            x_pre0 = load_x8(0)
            for j in range(1, npair):
                load_w8(j)
            qkv_block(*units[0], x_pre=x_pre0)
            if len(units) > 1:
                qkv_block(*units[1])
            for i in range(n_cores):
                nc.sync.dma_start(wp_sb[i][:], wproj[i * fl:(i + 1) * fl, :])
            last_b = bsz - 1
            # schedule hints: don't let the list scheduler hoist proj work
            # (which waits on its batch's collective) into the attention
            # stream before the collective can have finished.
            import os as _os
            _f = [float(v) for v in _os.environ.get(
                "PROJ_MS", "0.094,0.130,0.178,0,0.208,0.225").split(",")]
            for L, (b, qb) in enumerate(units):
                attn_block(b, qb)
                if L + 2 < len(units):
                    qkv_block(*units[L + 2])
                if b in split_batches and qb == nqb // 2 - 1:
                    a2a_issue_half(b, 0)
                if qb == nqb - 1:
                    if b in split_batches:
                        a2a_issue_half(b, 1)
                    else:
                        a2a_issue(b)
                    if b >= 1 and b - 1 not in split_batches:
                        proj(b - 1, _f[b - 1], after_attn=(b == last_b))
            if split_last:
                proj_half(last_b, 0, _f[4], after_attn=True)
                proj_half(last_b, 1, _f[5], after_attn=True)
            else:
                proj(bsz - 1, _f[5], after_attn=True)
    _split_multi_waits(nc)
    return nc


def _hi_lo(a, scale):
    """fp8 e4m3 residual pair of `scale*a`: a ~ (hi + lo)/scale."""
    f8 = mybir.dt.np(mybir.dt.float8e4)
    s = (scale * np.asarray(a, np.float32))
    hi = s.astype(f8)
    lo = (s - hi.astype(np.float32)).astype(f8)
    return np.ascontiguousarray(hi), np.ascontiguousarray(lo)


def shard_inputs(x, w_qkv, w_proj, n_cores=N_CORES, n_head=N_HEAD):
    bf16 = ml_dtypes.bfloat16
    d = x.shape[-1]
    T = x.shape[0] * x.shape[1]
    hpc = n_head // n_cores
    fl = hpc * HD
    xT = np.asarray(x, np.float32).reshape(T, d).T
    xh, xl = _hi_lo(xT, SX)
    wq = np.asarray(w_qkv, np.float32)
    wp = np.ascontiguousarray(np.asarray(w_proj, np.float32).T.astype(bf16))
    in_maps = []
    for c in range(n_cores):
        r0 = c * fl
        wqkv_c = np.concatenate(
            [wq[r0:r0 + fl], wq[d + r0:d + r0 + fl],
             wq[2 * d + r0:2 * d + r0 + fl]], axis=0).T
        wqh, wql = _hi_lo(wqkv_c, SW)
        in_maps.append({"xh": xh, "xl": xl, "wqh": wqh, "wql": wql,
                       "wproj": wp})
    return in_maps


def assemble_out(outs, n_cores=N_CORES, bsz=BSZ, seq=SEQ, d=D):
    """outs[c] is [d, bsz*tsb]; column block b holds tokens
    b*seq + [c*tsb, (c+1)*tsb) — except half-exchanged batches (the last two
    when nqb==4): tokens b*seq + h*seq/2 + [c*tsh, (c+1)*tsh)."""
    tsb = seq // n_cores
    tsh = tsb // 2
    split_last = (seq // min(512, seq)) == 4
    split_batches = {bsz - 1} if split_last else set()
    T = bsz * seq
    outT = np.empty((d, T), np.float32)
    for c in range(n_cores):
        for b in range(bsz):
            if b in split_batches:
                for h in range(2):
                    base = b * seq + h * (seq // 2)
                    outT[:, base + c * tsh:base + (c + 1) * tsh] = \
                        outs[c][:, b * tsb + h * tsh:b * tsb + (h + 1) * tsh]
            else:
                outT[:, b * seq + c * tsb:b * seq + (c + 1) * tsb] = \
                    outs[c][:, b * tsb:(b + 1) * tsb]
    return np.ascontiguousarray(outT.T).reshape(bsz, seq, d)


_NC_CACHE = {}


def kernel(x, w_qkv, w_proj):
    key = "full"
    if key not in _NC_CACHE:
        _NC_CACHE[key] = build_nc()
    nc = _NC_CACHE[key]
    in_maps = shard_inputs(x, w_qkv, w_proj)
    res = run_bass_kernel_spmd(nc, in_maps, list(range(N_CORES))).results
    return assemble_out([res[c]["out"] for c in range(N_CORES)]).astype(np.float32)
